# revision 7
# baseline (speedup 1.0000x reference)
"""Trainium2 Bass kernel for nn_Attention_35605278884484 (edge-augmented MHA).

B=1, N=512 nodes, H=8 heads, DH=64, DN=128 node feat, DE=64 edge feat.

Math (reference):
    q,k,v = nodes@W{q,k,v}+b ; e = edges@We+be (per head slices)
    sim[h,i,j] = scale * q[h,i] . (k[h,j] + e[h,i,j])
    attn = softmax_j(sim) ; out[h,i] = sum_j attn * (v[h,j] + e[h,i,j])
    final = concat_h(out) @ Wo + bo

Algebraic reductions used here (avoid materializing e, O(n^2 d_inner)):
    q.e[i,j]   = edges[i,j] . u[h,i],  u[h,i] = We_h @ q[h,i]
    (bk, be drop out of softmax: constant-in-j logit shifts)
    sum_j attn*e = (attn[i] @ edges[i]) @ We_h + be_h  (sum_j attn = 1)
    => be, bv fold into a host-side output bias (be+bv) @ Wo + bo

Sharding: sequence (i) sharded, 64 query rows per core; each core reads
only its (64, 512, 64) slice of edges (8MB) and computes all 8 heads.
Host concatenates the 8 (64, 128) results.

Device layout: "transposed" simT (j-partitions, (i,h) free) so that
- qk comes from kT/qT projections (strided psum writes per head),
- the edge logit term is one block-diag matmul per i-pair
  (lhsT = PE-transposed edge pair (128=2x64de, 128j), rhs = zero-padded
  u-pair (128, 16)) accumulating into the same psum bank,
- softmax denominators are a ones-matmul over j partitions.
"""
import os
import sys
import types
import contextlib
import ctypes

sys.path.insert(0, '/opt/trn_rl_repo')
sys.path.insert(0, '/root/.axon_site')

import numpy as np

H, DH = 8, 64
B, N, DN, DE = 1, 512, 128, 64
INNER = H * DH
NCORES = 8
NI = N // NCORES          # 64 query rows per core
NP = NI // 2              # 32 i-pairs per core
NJT = N // 128            # 4 j tiles
SCALE = float(DH ** -0.5)

_CACHE = {}


def _install_ntff_hook():
    """antenv.axon_hooks is absent in this image; synthesize it so
    run_bass_kernel_spmd(trace=True) can profile via libaxon."""
    if "antenv.axon_hooks" in sys.modules:
        return
    try:
        from trn_agent_boot.trn_boot import _ntff_profile_via_ctypes
        hook = _ntff_profile_via_ctypes('/opt/axon/libaxon_pjrt.so')
    except Exception:
        hook = None
    mod = types.ModuleType("antenv.axon_hooks")
    mod.get_axon_ntff_profile_hook = lambda: hook
    mod.set_axon_ntff_profile_hook = lambda h: None
    sys.modules["antenv.axon_hooks"] = mod


def _build():
    import concourse.mybir as mybir
    from concourse import bacc
    from concourse.tile import TileContext
    from concourse.masks import make_identity

    f32 = mybir.dt.float32
    nc = bacc.Bacc(None, target_bir_lowering=False)

    nodes_d = nc.declare_dram_parameter("nodes", [N, DN], f32, isOutput=False)
    nmy_d = nc.declare_dram_parameter("nodes_my", [NI, DN], f32, isOutput=False)
    edges_d = nc.declare_dram_parameter("edges_s", [NI, N, DE], f32, isOutput=False)
    wq_d = nc.declare_dram_parameter("Wq", [DN, INNER], f32, isOutput=False)
    wk_d = nc.declare_dram_parameter("Wk", [DN, INNER], f32, isOutput=False)
    wv_d = nc.declare_dram_parameter("Wv", [DN, INNER], f32, isOutput=False)
    we_d = nc.declare_dram_parameter("We", [DE, INNER], f32, isOutput=False)
    wo_d = nc.declare_dram_parameter("Wo", [INNER, DN], f32, isOutput=False)
    bq_d = nc.declare_dram_parameter("bq", [INNER], f32, isOutput=False)
    out_d = nc.declare_dram_parameter("out_my", [NI, DN], f32, isOutput=True)

    with TileContext(nc) as tc:
        with contextlib.ExitStack() as ctx:
            const = ctx.enter_context(tc.tile_pool(name="const", bufs=1))
            work = ctx.enter_context(tc.tile_pool(name="work", bufs=6))
            en_pool = ctx.enter_context(tc.tile_pool(name="edges", bufs=1))
            # PSUM budget is 8 banks total; every tile here is 1 bank.
            ps_eT = ctx.enter_context(tc.tile_pool(name="ps_eT", bufs=2, space="PSUM"))
            ps_simT = ctx.enter_context(tc.tile_pool(name="ps_simT", bufs=2, space="PSUM"))
            ps_misc = ctx.enter_context(tc.tile_pool(name="ps_misc", bufs=1, space="PSUM"))
            ps_acc = ctx.enter_context(tc.tile_pool(name="ps_acc", bufs=1, space="PSUM"))
            ps_w = ctx.enter_context(tc.tile_pool(name="ps_w", bufs=2, space="PSUM"))

            # ---- edge slice DMAs first so the queues run ahead ----
            en_t = [en_pool.tile([128, NI, DE], f32, tag=f"en{jt}", name=f"en{jt}")
                     for jt in range(NJT)]
            for jt in range(NJT):
                eng = nc.sync if jt < 2 else nc.scalar
                for blk in range(8):
                    eng.dma_start(
                        out=en_t[jt][:, blk * 8:(blk + 1) * 8, :],
                        in_=edges_d[blk * 8:(blk + 1) * 8, jt * 128:(jt + 1) * 128, :]
                        .rearrange("i j d -> j i d"))

            # ---- constants / weights ----
            ident = const.tile([128, 128], f32)
            make_identity(nc, ident)
            ones = const.tile([128, 128], f32)
            nc.vector.memset(ones, 1.0)

            wq_sb = const.tile([DN, INNER], f32)
            nc.gpsimd.dma_start(out=wq_sb, in_=wq_d[:, :])
            wk_sb = const.tile([DN, INNER], f32)
            nc.gpsimd.dma_start(out=wk_sb, in_=wk_d[:, :])
            wv_sb = const.tile([DN, INNER], f32)
            nc.gpsimd.dma_start(out=wv_sb, in_=wv_d[:, :])
            we_sb = const.tile([DE, INNER], f32)
            nc.gpsimd.dma_start(out=we_sb, in_=we_d[:, :])
            wo_sb = const.tile([128, 4, DN], f32)
            nc.gpsimd.dma_start(out=wo_sb, in_=wo_d[:, :].rearrange("(c p) d -> p c d", p=128))
            bq_sb = const.tile([DH, H], f32)
            nc.gpsimd.dma_start(out=bq_sb, in_=bq_d[:].rearrange("(h d) -> d h", h=H))
            nodes_sb = const.tile([128, 4, DN], f32)
            nc.gpsimd.dma_start(out=nodes_sb, in_=nodes_d[:, :].rearrange("(t p) d -> p t d", p=128))
            nmy_sb = const.tile([NI, DN], f32)
            nc.gpsimd.dma_start(out=nmy_sb, in_=nmy_d[:, :])

            # ---- nodesT (128 dn, 512 n) via PE transposes ----
            nodesT = const.tile([DN, N], f32)
            for t in range(4):
                pt = ps_misc.tile([128, 128], f32, tag="m", name="pt")
                nc.tensor.matmul(out=pt, lhsT=nodes_sb[:, t, :], rhs=ident,
                                 start=True, stop=True, is_transpose=True)
                nc.vector.tensor_copy(out=nodesT[:, t * 128:(t + 1) * 128], in_=pt)
            # nodes_myT (128 dn, 64 i)
            nmyT = const.tile([DN, NI], f32)
            pt = ps_misc.tile([128, 128], f32, tag="m", name="pt")
            nc.tensor.matmul(out=pt[:, 0:NI], lhsT=nmy_sb, rhs=ident[0:NI, 0:NI],
                             start=True, stop=True, is_transpose=True)
            nc.vector.tensor_copy(out=nmyT, in_=pt[:, 0:NI])
            # WeT (64 dh, 8, 64 de)
            weT = const.tile([DH, H, DE], f32)
            for h in range(H):
                pt = ps_misc.tile([128, 128], f32, tag="m", name="pt")
                nc.tensor.matmul(out=pt[0:DH, 0:DE], lhsT=we_sb[:, h * DH:(h + 1) * DH],
                                 rhs=ident[0:DE, 0:DE], start=True, stop=True, is_transpose=True)
                nc.vector.tensor_copy(out=weT[:, h, :], in_=pt[0:DH, 0:DE])

            # ---- projections ----
            kT = const.tile([DH, H, N], f32)          # (dh, h, j)
            for h in range(H):
                pk = ps_misc.tile([DH, N], f32, tag="m", name="pk")
                nc.tensor.matmul(out=pk, lhsT=wk_sb[:, h * DH:(h + 1) * DH], rhs=nodesT,
                                 start=True, stop=True)
                nc.vector.tensor_copy(out=kT[:, h, :], in_=pk)
            qT = const.tile([DH, H, NI], f32)         # (dh, h, i) with bq added
            for h in range(H):
                pq = ps_misc.tile([DH, NI], f32, tag="m", name="pq")
                nc.tensor.matmul(out=pq, lhsT=wq_sb[:, h * DH:(h + 1) * DH], rhs=nmyT,
                                 start=True, stop=True)
                nc.vector.tensor_scalar_add(out=qT[:, h, :], in0=pq, scalar1=bq_sb[:, h:h + 1])
            v4 = const.tile([128, NJT, INNER], f32)   # (j, jt, (h dh))
            for jt in range(NJT):
                pv = ps_misc.tile([128, INNER], f32, tag="m", name="pv")
                nc.tensor.matmul(out=pv, lhsT=nodesT[:, jt * 128:(jt + 1) * 128], rhs=wv_sb,
                                 start=True, stop=True)
                nc.vector.tensor_copy(out=v4[:, jt, :], in_=pv)
            # u2: paired block-diag u (128 = 2x64 de, 32 pairs, 16 = 2x8 h)
            u2 = const.tile([128, NP, 16], f32)
            nc.gpsimd.memset(u2, 0.0)
            for h in range(H):
                pu = ps_misc.tile([DE, NI], f32, tag="m", name="pu")
                nc.tensor.matmul(out=pu, lhsT=weT[:, h, :], rhs=qT[:, h, :],
                                 start=True, stop=True)
                puv = pu.rearrange("d (p two) -> d p two", two=2)
                nc.vector.tensor_copy(out=u2[0:DE, :, h], in_=puv[:, :, 0])
                nc.vector.tensor_copy(out=u2[DE:128, :, 8 + h], in_=puv[:, :, 1])

            # ---- logits simT (j, i, h) per j-tile; exp ----
            expT = const.tile([128, NJT, NI * H], f32)
            for jt in range(NJT):
                simT = ps_simT.tile([128, NI, H], f32, tag="simT", name="simT")
                simv = simT
                for h in range(H):
                    nc.tensor.matmul(out=simv[:, :, h],
                                     lhsT=kT[:, h, jt * 128:(jt + 1) * 128],
                                     rhs=qT[:, h, :],
                                     start=(h == 0), stop=False, skip_group_check=True)
                for ip in range(NP):
                    peT = ps_eT.tile([128, 128], f32, tag="peT", name="peT")
                    nc.tensor.matmul(out=peT, lhsT=en_t[jt][:, 2 * ip:2 * ip + 2, :],
                                     rhs=ident, start=True, stop=True, is_transpose=True)
                    eT_sb = work.tile([128, 128], f32, tag="eT")
                    if ip % 2 == 0:
                        nc.vector.tensor_copy(out=eT_sb, in_=peT)
                    else:
                        nc.scalar.copy(out=eT_sb, in_=peT)
                    nc.tensor.matmul(out=simv[:, 2 * ip:2 * ip + 2, :],
                                     lhsT=eT_sb, rhs=u2[:, ip, :],
                                     start=False, stop=(ip == NP - 1), skip_group_check=True)
                nc.scalar.activation(out=expT[:, jt, :], in_=simv[:, :, :],
                                     func=mybir.ActivationFunctionType.Exp, scale=SCALE)

            # ---- softmax denominators + attnT ----
            den = ps_misc.tile([128, NI * H], f32, tag="m", name="den")
            for jt in range(NJT):
                nc.tensor.matmul(out=den, lhsT=ones, rhs=expT[:, jt, :],
                                 start=(jt == 0), stop=(jt == NJT - 1), skip_group_check=True)
            recip = const.tile([128, NI * H], f32)
            nc.vector.reciprocal(out=recip, in_=den)
            attnT = const.tile([128, NJT, NI * H], f32)
            for jt in range(NJT):
                (nc.vector if jt % 2 == 0 else nc.gpsimd).tensor_mul(
                    out=attnT[:, jt, :], in0=expT[:, jt, :], in1=recip)

            attv = attnT.rearrange("p t (i h) -> p t i h", h=H)

            # ---- out = attn @ v  (+ attn-edge term) into one psum bank ----
            pout = ps_acc.tile([DH, H, NI], f32)
            for h in range(H):
                for jt in range(NJT):
                    nc.tensor.matmul(out=pout[:, h, :],
                                     lhsT=v4[:, jt, h * DH:(h + 1) * DH],
                                     rhs=attv[:, jt, :, h],
                                     start=(h == 0 and jt == 0), stop=False,
                                     skip_group_check=True)

            # ---- w[i] = attn[i] @ edges[i]  (64 de, i, h), octets of i ----
            w_sb = const.tile([DE, NI, H], f32)
            for o in range(8):
                pw = ps_w.tile([DE, 8, H], f32, tag="pw")
                for ii in range(8):
                    i = o * 8 + ii
                    jt_list = range(NJT)
                    for jt in jt_list:
                        nc.tensor.matmul(out=pw[:, ii, :],
                                         lhsT=en_t[jt][:, i, :],
                                         rhs=attv[:, jt, i, :],
                                         start=(ii == 0 and jt == 0),
                                         stop=(ii == 7 and jt == NJT - 1),
                                         skip_group_check=True)
                nc.vector.tensor_copy(out=w_sb[:, o * 8:(o + 1) * 8, :], in_=pw)

            # ---- out_e: (w @ We_h) accumulated into pout ----
            for h in range(H):
                for o in range(8):
                    nc.tensor.matmul(out=pout[:, h, o * 8:(o + 1) * 8],
                                     lhsT=we_sb[:, h * DH:(h + 1) * DH],
                                     rhs=w_sb[:, o * 8:(o + 1) * 8, h],
                                     start=False,
                                     stop=(h == H - 1 and o == 7),
                                     skip_group_check=True)

            # ---- final projection ----
            oiT = const.tile([128, 4, NI], f32)       # ((h dh) chunk, c, i)
            for h in range(H):
                nc.vector.tensor_copy(out=oiT[(h % 2) * DH:(h % 2) * DH + DH, h // 2, :],
                                      in_=pout[:, h, :])
            pfin = ps_misc.tile([DN, NI], f32, tag="m", name="pfin")
            for c in range(4):
                nc.tensor.matmul(out=pfin, lhsT=wo_sb[:, c, :], rhs=oiT[:, c, :],
                                 start=(c == 0), stop=(c == 3), skip_group_check=True)
            fin_sb = const.tile([DN, NI], f32)
            nc.vector.tensor_copy(out=fin_sb, in_=pfin)
            pft = ps_misc.tile([NI, DN], f32, tag="m", name="pft")
            nc.tensor.matmul(out=pft, lhsT=fin_sb, rhs=ident, start=True, stop=True,
                             is_transpose=True)
            out_sb = const.tile([NI, DN], f32)
            nc.vector.tensor_copy(out=out_sb, in_=pft)
            nc.gpsimd.dma_start(out=out_d[:, :], in_=out_sb)

    nc.finalize()
    return nc


def kernel(nodes, edges, mask, Wq, bq, Wk, bk, Wv, bv, We, be, Wo, bo):
    from concourse.bass_utils import run_bass_kernel_spmd

    nodes = np.asarray(nodes, np.float32)
    edges = np.asarray(edges, np.float32)
    mask = np.asarray(mask)
    Wq = np.asarray(Wq, np.float32); bq = np.asarray(bq, np.float32)
    Wk = np.asarray(Wk, np.float32)
    Wv = np.asarray(Wv, np.float32); bv = np.asarray(bv, np.float32)
    We = np.asarray(We, np.float32); be = np.asarray(be, np.float32)
    Wo = np.asarray(Wo, np.float32); bo = np.asarray(bo, np.float32)
    assert mask.all(), "kernel assumes an all-true mask (spec fill=ones)"

    if "nc" not in _CACHE:
        _CACHE["nc"] = _build()
    nc = _CACHE["nc"]

    n0 = nodes[0]
    in_maps = []
    for c in range(NCORES):
        in_maps.append({
            "nodes": n0,
            "nodes_my": n0[c * NI:(c + 1) * NI],
            "edges_s": edges[0, c * NI:(c + 1) * NI],
            "Wq": Wq, "Wk": Wk, "Wv": Wv, "We": We, "Wo": Wo, "bq": bq,
        })

    trace = bool(os.environ.get("BASS_KERNEL_TRACE"))
    kw = {}
    if trace:
        _install_ntff_hook()
        import concourse.bass_utils as bu
        bu.upload_artifacts = lambda tmpdir: "local://skipped"
        kw = dict(trace=True, tmpdir=os.environ.get("BASS_KERNEL_TRACE_DIR") or None)
    res = run_bass_kernel_spmd(nc, in_maps, list(range(NCORES)), **kw)
    _CACHE["last_exec_ns"] = res.exec_time_ns

    out = np.concatenate([res.results[c]["out_my"] for c in range(NCORES)], axis=0)
    out = out + ((be + bv) @ Wo + bo)[None, :]
    return out.reshape(B, N, DN).astype(np.float32)


# revision 9
# speedup vs baseline: 1.9132x; 1.9132x over previous
"""Trainium2 Bass kernel for nn_Attention_35605278884484 (edge-augmented MHA).

B=1, N=512 nodes, H=8 heads, DH=64, DN=128 node feat, DE=64 edge feat.

Math (reference):
    q,k,v = nodes@W{q,k,v}+b ; e = edges@We+be (per head slices)
    sim[h,i,j] = scale * q[h,i] . (k[h,j] + e[h,i,j])
    attn = softmax_j(sim) ; out[h,i] = sum_j attn * (v[h,j] + e[h,i,j])
    final = concat_h(out) @ Wo + bo

Algebraic reductions used here (avoid materializing e, O(n^2 d_inner)):
    q.e[i,j]   = edges[i,j] . u[h,i],  u[h,i] = We_h @ q[h,i]
    (bk, be drop out of softmax: constant-in-j logit shifts)
    sum_j attn*e = (attn[i] @ edges[i]) @ We_h + be_h  (sum_j attn = 1)
    => be, bv fold into a host-side output bias (be+bv) @ Wo + bo

Sharding: sequence (i) sharded, 64 query rows per core; each core reads
only its (64, 512, 64) slice of edges (cast to bf16 on host) and
computes all 8 heads. Host concatenates the 8 (64, 128) results.

Device layout: "transposed" simT (j-partitions, (i,h) free):
- qk from kT/qT projections (strided psum writes per head),
- edge logit term: one block-diag matmul per i-pair
  (lhsT = PE-transposed edge pair (128=2x64de, 128j), rhs = zero-padded
  u-pair (128, 16)) accumulating into the same psum bank,
- softmax denominators via ones-matmul over j partitions,
- attn-edge output term w = attn @ edges also i-pair-batched (off-diag
  blocks are discarded garbage).
All PE operands are bf16 (fp32 matmul runs double-pass LOW_HIGH on
trn2); accumulations stay fp32 in PSUM.
"""
import os
import sys
import types
import contextlib

sys.path.insert(0, '/opt/trn_rl_repo')
sys.path.insert(0, '/root/.axon_site')

import numpy as np
import ml_dtypes

H, DH = 8, 64
B, N, DN, DE = 1, 512, 128, 64
INNER = H * DH
NCORES = 8
NI = N // NCORES          # 64 query rows per core
NP = NI // 2              # 32 i-pairs per core
NJT = N // 128            # 4 j tiles
SCALE = float(DH ** -0.5)
BF16 = ml_dtypes.bfloat16

_CACHE = {}


def _install_ntff_hook():
    """antenv.axon_hooks is absent in this image; synthesize it so
    run_bass_kernel_spmd(trace=True) can profile via libaxon."""
    if "antenv.axon_hooks" in sys.modules:
        return
    try:
        from trn_agent_boot.trn_boot import _ntff_profile_via_ctypes
        hook = _ntff_profile_via_ctypes('/opt/axon/libaxon_pjrt.so')
    except Exception:
        hook = None
    mod = types.ModuleType("antenv.axon_hooks")
    mod.get_axon_ntff_profile_hook = lambda: hook
    mod.set_axon_ntff_profile_hook = lambda h: None
    sys.modules["antenv.axon_hooks"] = mod


def _build():
    import concourse.mybir as mybir
    from concourse import bacc
    from concourse.tile import TileContext
    from concourse.masks import make_identity

    f32 = mybir.dt.float32
    bf = mybir.dt.bfloat16
    nc = bacc.Bacc(None, target_bir_lowering=False)

    nodes_d = nc.declare_dram_parameter("nodes", [N, DN], bf, isOutput=False)
    nmy_d = nc.declare_dram_parameter("nodes_my", [NI, DN], bf, isOutput=False)
    edges_d = nc.declare_dram_parameter("edges_s", [NI, N, DE], bf, isOutput=False)
    wq_d = nc.declare_dram_parameter("Wq", [DN, INNER], bf, isOutput=False)
    wk_d = nc.declare_dram_parameter("Wk", [DN, INNER], bf, isOutput=False)
    wv_d = nc.declare_dram_parameter("Wv", [DN, INNER], bf, isOutput=False)
    we_d = nc.declare_dram_parameter("We", [DE, INNER], bf, isOutput=False)
    wo_d = nc.declare_dram_parameter("Wo", [INNER, DN], bf, isOutput=False)
    bq_d = nc.declare_dram_parameter("bq", [INNER], f32, isOutput=False)
    out_d = nc.declare_dram_parameter("out_my", [NI, DN], f32, isOutput=True)

    with TileContext(nc) as tc:
        with contextlib.ExitStack() as ctx:
            const = ctx.enter_context(tc.tile_pool(name="const", bufs=1))
            work = ctx.enter_context(tc.tile_pool(name="work", bufs=6))
            en_pool = ctx.enter_context(tc.tile_pool(name="edges", bufs=1))
            # PSUM budget is 8 banks; every tile below pads to 1 bank.
            ps_eT = ctx.enter_context(tc.tile_pool(name="ps_eT", bufs=2, space="PSUM"))
            ps_simT = ctx.enter_context(tc.tile_pool(name="ps_simT", bufs=2, space="PSUM"))
            ps_misc = ctx.enter_context(tc.tile_pool(name="ps_misc", bufs=1, space="PSUM"))
            ps_acc = ctx.enter_context(tc.tile_pool(name="ps_acc", bufs=1, space="PSUM"))
            ps_w = ctx.enter_context(tc.tile_pool(name="ps_w", bufs=2, space="PSUM"))

            # ---- edge slice DMAs first so the queues run ahead ----
            en_t = [en_pool.tile([128, NI, DE], bf, tag=f"en{jt}", name=f"en{jt}")
                    for jt in range(NJT)]
            for jt in range(NJT):
                eng = nc.sync if jt < 2 else nc.scalar
                for blk in range(8):
                    eng.dma_start(
                        out=en_t[jt][:, blk * 8:(blk + 1) * 8, :],
                        in_=edges_d[blk * 8:(blk + 1) * 8, jt * 128:(jt + 1) * 128, :]
                        .rearrange("i j d -> j i d"))

            # ---- constants / weights ----
            ident = const.tile([128, 128], bf)
            make_identity(nc, ident)
            ones = const.tile([128, 128], bf)
            nc.vector.memset(ones, 1.0)

            wq_sb = const.tile([DN, INNER], bf)
            nc.gpsimd.dma_start(out=wq_sb, in_=wq_d[:, :])
            wk_sb = const.tile([DN, INNER], bf)
            nc.gpsimd.dma_start(out=wk_sb, in_=wk_d[:, :])
            wv_sb = const.tile([DN, INNER], bf)
            nc.gpsimd.dma_start(out=wv_sb, in_=wv_d[:, :])
            we_sb = const.tile([DE, INNER], bf)
            nc.gpsimd.dma_start(out=we_sb, in_=we_d[:, :])
            wo_sb = const.tile([128, 4, DN], bf)
            nc.gpsimd.dma_start(out=wo_sb, in_=wo_d[:, :].rearrange("(c p) d -> p c d", p=128))
            bq_sb = const.tile([DH, H], f32)
            nc.gpsimd.dma_start(out=bq_sb, in_=bq_d[:].rearrange("(h d) -> d h", h=H))
            nodes_sb = const.tile([128, 4, DN], bf)
            nc.gpsimd.dma_start(out=nodes_sb, in_=nodes_d[:, :].rearrange("(t p) d -> p t d", p=128))
            nmy_sb = const.tile([NI, DN], bf)
            nc.gpsimd.dma_start(out=nmy_sb, in_=nmy_d[:, :])

            # ---- nodesT (128 dn, 512 n) via PE transposes ----
            nodesT = const.tile([DN, N], bf)
            for t in range(4):
                pt = ps_misc.tile([128, 128], bf, tag="m", name="pt")
                nc.tensor.matmul(out=pt, lhsT=nodes_sb[:, t, :], rhs=ident,
                                 start=True, stop=True, is_transpose=True)
                nc.vector.tensor_copy(out=nodesT[:, t * 128:(t + 1) * 128], in_=pt)
            # nodes_myT (128 dn, 64 i)
            nmyT = const.tile([DN, NI], bf)
            pt = ps_misc.tile([128, 128], bf, tag="m", name="pt2")
            nc.tensor.matmul(out=pt[:, 0:NI], lhsT=nmy_sb, rhs=ident[0:NI, 0:NI],
                             start=True, stop=True, is_transpose=True)
            nc.vector.tensor_copy(out=nmyT, in_=pt[:, 0:NI])
            # WeT (64 dh, 8, 64 de)
            weT = const.tile([DH, H, DE], bf)
            for h in range(H):
                pt = ps_misc.tile([128, 128], bf, tag="m", name="pt3")
                nc.tensor.matmul(out=pt[0:DH, 0:DE], lhsT=we_sb[:, h * DH:(h + 1) * DH],
                                 rhs=ident[0:DE, 0:DE], start=True, stop=True, is_transpose=True)
                nc.vector.tensor_copy(out=weT[:, h, :], in_=pt[0:DH, 0:DE])

            # ---- projections ----
            kT = const.tile([DH, H, N], bf)           # (dh, h, j)
            for h in range(H):
                pk = ps_misc.tile([DH, N], f32, tag="m", name="pk")
                nc.tensor.matmul(out=pk, lhsT=wk_sb[:, h * DH:(h + 1) * DH], rhs=nodesT,
                                 start=True, stop=True)
                nc.vector.tensor_copy(out=kT[:, h, :], in_=pk)
            qT = const.tile([DH, H, NI], bf)          # (dh, h, i) with bq added
            for h in range(H):
                pq = ps_misc.tile([DH, NI], f32, tag="m", name="pq")
                nc.tensor.matmul(out=pq, lhsT=wq_sb[:, h * DH:(h + 1) * DH], rhs=nmyT,
                                 start=True, stop=True)
                nc.vector.tensor_scalar_add(out=qT[:, h, :], in0=pq, scalar1=bq_sb[:, h:h + 1])
            v4 = const.tile([128, NJT, INNER], bf)    # (j, jt, (h dh))
            for jt in range(NJT):
                pv = ps_misc.tile([128, INNER], f32, tag="m", name="pv")
                nc.tensor.matmul(out=pv, lhsT=nodesT[:, jt * 128:(jt + 1) * 128], rhs=wv_sb,
                                 start=True, stop=True)
                nc.vector.tensor_copy(out=v4[:, jt, :], in_=pv)
            # u2: paired block-diag u (128 = 2x64 de, 32 pairs, 16 = 2x8 h)
            u2 = const.tile([128, NP, 16], bf)
            nc.gpsimd.memset(u2, 0.0)
            for h in range(H):
                pu = ps_misc.tile([DE, NI], f32, tag="m", name="pu")
                nc.tensor.matmul(out=pu, lhsT=weT[:, h, :], rhs=qT[:, h, :],
                                 start=True, stop=True)
                puv = pu.rearrange("d (p two) -> d p two", two=2)
                nc.vector.tensor_copy(out=u2[0:DE, :, h], in_=puv[:, :, 0])
                nc.vector.tensor_copy(out=u2[DE:128, :, 8 + h], in_=puv[:, :, 1])

            # ---- logits simT (j, i, h) per j-tile; exp ----
            expT = const.tile([128, NJT, NI * H], bf)
            for jt in range(NJT):
                simT = ps_simT.tile([128, NI, H], f32, tag="simT", name="simT")
                for h in range(H):
                    nc.tensor.matmul(out=simT[:, :, h],
                                     lhsT=kT[:, h, jt * 128:(jt + 1) * 128],
                                     rhs=qT[:, h, :],
                                     start=(h == 0), stop=False, skip_group_check=True)
                for ip in range(NP):
                    peT = ps_eT.tile([128, 128], bf, tag="peT", name="peT")
                    nc.tensor.matmul(out=peT, lhsT=en_t[jt][:, 2 * ip:2 * ip + 2, :],
                                     rhs=ident, start=True, stop=True, is_transpose=True)
                    eT_sb = work.tile([128, 128], bf, tag="eT")
                    if ip % 2 == 0:
                        nc.vector.tensor_copy(out=eT_sb, in_=peT)
                    else:
                        nc.scalar.copy(out=eT_sb, in_=peT)
                    nc.tensor.matmul(out=simT[:, 2 * ip:2 * ip + 2, :],
                                     lhsT=eT_sb, rhs=u2[:, ip, :],
                                     start=False, stop=(ip == NP - 1), skip_group_check=True)
                nc.scalar.activation(out=expT[:, jt, :], in_=simT[:, :, :],
                                     func=mybir.ActivationFunctionType.Exp, scale=SCALE)

            # ---- softmax denominators + attnT ----
            den = ps_misc.tile([128, NI * H], f32, tag="m", name="den")
            for jt in range(NJT):
                nc.tensor.matmul(out=den, lhsT=ones, rhs=expT[:, jt, :],
                                 start=(jt == 0), stop=(jt == NJT - 1), skip_group_check=True)
            recip = const.tile([128, NI * H], f32)
            nc.vector.reciprocal(out=recip, in_=den)
            recip_bf = const.tile([128, NI * H], bf)
            nc.vector.tensor_copy(out=recip_bf, in_=recip)
            attnT = const.tile([128, NJT, NI * H], bf)
            for jt in range(NJT):
                (nc.vector if jt % 2 == 0 else nc.gpsimd).tensor_mul(
                    out=attnT[:, jt, :], in0=expT[:, jt, :], in1=recip_bf)

            attv = attnT.rearrange("p t (i h) -> p t i h", h=H)

            # ---- out = attn @ v into one psum bank ----
            pout = ps_acc.tile([DH, H, NI], f32)
            for h in range(H):
                for jt in range(NJT):
                    nc.tensor.matmul(out=pout[:, h, :],
                                     lhsT=v4[:, jt, h * DH:(h + 1) * DH],
                                     rhs=attv[:, jt, :, h],
                                     start=(h == 0 and jt == 0), stop=False,
                                     skip_group_check=True)

            # ---- w[i] = attn[i] @ edges[i]  (pair-batched; off-diag garbage) ----
            w_sb = const.tile([DE, NI, H], bf)
            for ip in range(NP):
                pw = ps_w.tile([128, 16], f32, tag="pw", name="pw")
                for jt in range(NJT):
                    nc.tensor.matmul(out=pw,
                                     lhsT=en_t[jt][:, 2 * ip:2 * ip + 2, :],
                                     rhs=attv[:, jt, 2 * ip:2 * ip + 2, :],
                                     start=(jt == 0), stop=(jt == NJT - 1),
                                     skip_group_check=True)
                nc.vector.tensor_copy(out=w_sb[:, 2 * ip, :], in_=pw[0:DE, 0:8])
                nc.vector.tensor_copy(out=w_sb[:, 2 * ip + 1, :], in_=pw[DE:128, 8:16])

            # ---- out_e: (w @ We_h) accumulated into pout ----
            for h in range(H):
                nc.tensor.matmul(out=pout[:, h, :],
                                 lhsT=we_sb[:, h * DH:(h + 1) * DH],
                                 rhs=w_sb[:, :, h],
                                 start=False, stop=(h == H - 1),
                                 skip_group_check=True)

            # ---- final projection ----
            oiT = const.tile([128, 4, NI], bf)        # ((h dh) chunk, c, i)
            for h in range(H):
                nc.vector.tensor_copy(out=oiT[(h % 2) * DH:(h % 2) * DH + DH, h // 2, :],
                                      in_=pout[:, h, :])
            pfin = ps_misc.tile([DN, NI], f32, tag="m", name="pfin")
            for c in range(4):
                nc.tensor.matmul(out=pfin, lhsT=wo_sb[:, c, :], rhs=oiT[:, c, :],
                                 start=(c == 0), stop=(c == 3), skip_group_check=True)
            fin_sb = const.tile([DN, NI], bf)
            nc.vector.tensor_copy(out=fin_sb, in_=pfin)
            pft = ps_misc.tile([NI, DN], bf, tag="m", name="pft")
            nc.tensor.matmul(out=pft, lhsT=fin_sb, rhs=ident, start=True, stop=True,
                             is_transpose=True)
            out_sb = const.tile([NI, DN], f32)
            nc.vector.tensor_copy(out=out_sb, in_=pft)
            nc.gpsimd.dma_start(out=out_d[:, :], in_=out_sb)

    nc.finalize()
    return nc


def kernel(nodes, edges, mask, Wq, bq, Wk, bk, Wv, bv, We, be, Wo, bo):
    from concourse.bass_utils import run_bass_kernel_spmd

    nodes = np.asarray(nodes, np.float32)
    edges = np.asarray(edges, np.float32)
    mask = np.asarray(mask)
    Wq = np.asarray(Wq, np.float32); bq = np.asarray(bq, np.float32)
    Wk = np.asarray(Wk, np.float32)
    Wv = np.asarray(Wv, np.float32); bv = np.asarray(bv, np.float32)
    We = np.asarray(We, np.float32); be = np.asarray(be, np.float32)
    Wo = np.asarray(Wo, np.float32); bo = np.asarray(bo, np.float32)
    assert mask.all(), "kernel assumes an all-true mask (spec fill=ones)"

    if "nc" not in _CACHE:
        _CACHE["nc"] = _build()
    nc = _CACHE["nc"]

    n0 = nodes[0].astype(BF16)
    e_bf = edges[0].astype(BF16)
    wq_b = Wq.astype(BF16); wk_b = Wk.astype(BF16); wv_b = Wv.astype(BF16)
    we_b = We.astype(BF16); wo_b = Wo.astype(BF16)
    in_maps = []
    for c in range(NCORES):
        in_maps.append({
            "nodes": n0,
            "nodes_my": n0[c * NI:(c + 1) * NI],
            "edges_s": e_bf[c * NI:(c + 1) * NI],
            "Wq": wq_b, "Wk": wk_b, "Wv": wv_b, "We": we_b, "Wo": wo_b, "bq": bq,
        })

    trace = bool(os.environ.get("BASS_KERNEL_TRACE"))
    kw = {}
    if trace:
        _install_ntff_hook()
        import concourse.bass_utils as bu
        bu.upload_artifacts = lambda tmpdir: "local://skipped"
        kw = dict(trace=True, tmpdir=os.environ.get("BASS_KERNEL_TRACE_DIR") or None)
    res = run_bass_kernel_spmd(nc, in_maps, list(range(NCORES)), **kw)
    _CACHE["last_exec_ns"] = res.exec_time_ns

    out = np.concatenate([res.results[c]["out_my"] for c in range(NCORES)], axis=0)
    out = out + ((be + bv) @ Wo + bo)[None, :]
    return out.reshape(B, N, DN).astype(np.float32)


# revision 10
# speedup vs baseline: 3.3111x; 1.7307x over previous
"""Trainium2 Bass kernel for nn_Attention_35605278884484 (edge-augmented MHA).

B=1, N=512 nodes, H=8 heads, DH=64, DN=128 node feat, DE=64 edge feat.

Math (reference):
    q,k,v = nodes@W{q,k,v}+b ; e = edges@We+be (per head slices)
    sim[h,i,j] = scale * q[h,i] . (k[h,j] + e[h,i,j])
    attn = softmax_j(sim) ; out[h,i] = sum_j attn * (v[h,j] + e[h,i,j])
    final = concat_h(out) @ Wo + bo

Algebraic reductions used here (avoid materializing e, O(n^2 d_inner)):
    q.e[i,j]   = edges[i,j] . u[h,i],  u[h,i] = We_h @ q[h,i]
    (bk, be drop out of softmax: constant-in-j logit shifts)
    sum_j attn*e = (attn[i] @ edges[i]) @ We_h + be_h  (sum_j attn = 1)
    => be, bv fold into a host-side output bias (be+bv) @ Wo + bo

Sharding: sequence (i) sharded, 64 query rows per core; each core reads
only its (64, 512, 64) slice of edges (cast to bf16 on host) and
computes all 8 heads. Host concatenates the 8 (64, 128) results.

Device layout: "transposed" simT (j-partitions, (i,h) free):
- qk from kT/qT projections (strided psum writes per head),
- edge logit term: one block-diag matmul per i-pair
  (lhsT = PE-transposed edge pair (128=2x64de, 128j), rhs = zero-padded
  u-pair (128, 16)) accumulating into the same psum bank,
- softmax denominators via ones-matmul over j partitions,
- attn-edge output term w = attn @ edges also i-pair-batched (off-diag
  blocks are discarded garbage).
All PE operands are bf16 (fp32 matmul runs double-pass LOW_HIGH on
trn2); accumulations stay fp32 in PSUM.
"""
import os
import sys
import types
import contextlib

sys.path.insert(0, '/opt/trn_rl_repo')
sys.path.insert(0, '/root/.axon_site')

import numpy as np
import ml_dtypes

H, DH = 8, 64
B, N, DN, DE = 1, 512, 128, 64
INNER = H * DH
NCORES = 8
NI = N // NCORES          # 64 query rows per core
NP = NI // 2              # 32 i-pairs per core
NJT = N // 128            # 4 j tiles
SCALE = float(DH ** -0.5)
BF16 = ml_dtypes.bfloat16

_CACHE = {}


def _install_ntff_hook():
    """antenv.axon_hooks is absent in this image; synthesize it so
    run_bass_kernel_spmd(trace=True) can profile via libaxon."""
    if "antenv.axon_hooks" in sys.modules:
        return
    try:
        from trn_agent_boot.trn_boot import _ntff_profile_via_ctypes
        hook = _ntff_profile_via_ctypes('/opt/axon/libaxon_pjrt.so')
    except Exception:
        hook = None
    mod = types.ModuleType("antenv.axon_hooks")
    mod.get_axon_ntff_profile_hook = lambda: hook
    mod.set_axon_ntff_profile_hook = lambda h: None
    sys.modules["antenv.axon_hooks"] = mod


def _build():
    import concourse.mybir as mybir
    from concourse import bacc
    from concourse.tile import TileContext
    from concourse.masks import make_identity

    f32 = mybir.dt.float32
    bf = mybir.dt.bfloat16
    nc = bacc.Bacc(None, target_bir_lowering=False)

    nodes_d = nc.declare_dram_parameter("nodes", [N, DN], bf, isOutput=False)
    nmy_d = nc.declare_dram_parameter("nodes_my", [NI, DN], bf, isOutput=False)
    enat_d = nc.declare_dram_parameter("edges_nat", [N, NI, DE], bf, isOutput=False)
    etr_d = nc.declare_dram_parameter("edges_T", [NI, DE, N], bf, isOutput=False)
    wq_d = nc.declare_dram_parameter("Wq", [DN, INNER], bf, isOutput=False)
    wk_d = nc.declare_dram_parameter("Wk", [DN, INNER], bf, isOutput=False)
    wv_d = nc.declare_dram_parameter("Wv", [DN, INNER], bf, isOutput=False)
    we_d = nc.declare_dram_parameter("We", [DE, INNER], bf, isOutput=False)
    wo_d = nc.declare_dram_parameter("Wo", [INNER, DN], bf, isOutput=False)
    bq_d = nc.declare_dram_parameter("bq", [INNER], f32, isOutput=False)
    out_d = nc.declare_dram_parameter("out_my", [NI, DN], f32, isOutput=True)

    with TileContext(nc) as tc:
        with contextlib.ExitStack() as ctx:
            const = ctx.enter_context(tc.tile_pool(name="const", bufs=1))
            en_pool = ctx.enter_context(tc.tile_pool(name="edges", bufs=1))
            # PSUM budget is 8 banks; every tile below pads to 1 bank.
            ps_simT = ctx.enter_context(tc.tile_pool(name="ps_simT", bufs=3, space="PSUM"))
            ps_misc = ctx.enter_context(tc.tile_pool(name="ps_misc", bufs=2, space="PSUM"))
            ps_acc = ctx.enter_context(tc.tile_pool(name="ps_acc", bufs=1, space="PSUM"))
            ps_w = ctx.enter_context(tc.tile_pool(name="ps_w", bufs=2, space="PSUM"))

            # ---- edge slice DMAs first so the queues run ahead ----
            # eT_big: (128 = (i-parity, de), pair, j) -- lhsT tiles for the
            # block-diag logit matmuls. Fully contiguous (1KB runs).
            eT_big = en_pool.tile([128, NP, N], bf, tag="eTb", name="eT_big")
            etr_v = etr_d[:, :, :].rearrange("(g two) d j -> (two d) g j", two=2)
            for g in range(4):
                nc.scalar.dma_start(out=eT_big[:, g * 8:(g + 1) * 8, :],
                                    in_=etr_v[:, g * 8:(g + 1) * 8, :])
            # e_nat: (j, i, de) per j-tile for the w contraction. Contiguous.
            en_t = [en_pool.tile([128, NI, DE], bf, tag=f"en{jt}", name=f"en{jt}")
                    for jt in range(NJT)]
            for jt in range(NJT):
                nc.sync.dma_start(out=en_t[jt],
                                  in_=enat_d[jt * 128:(jt + 1) * 128, :, :])

            # ---- constants / weights ----
            ident = const.tile([128, 128], bf)
            make_identity(nc, ident)
            ones = const.tile([128, 128], bf)
            nc.vector.memset(ones, 1.0)

            wq_sb = const.tile([DN, INNER], bf)
            nc.gpsimd.dma_start(out=wq_sb, in_=wq_d[:, :])
            wk_sb = const.tile([DN, INNER], bf)
            nc.gpsimd.dma_start(out=wk_sb, in_=wk_d[:, :])
            wv_sb = const.tile([DN, INNER], bf)
            nc.gpsimd.dma_start(out=wv_sb, in_=wv_d[:, :])
            we_sb = const.tile([DE, INNER], bf)
            nc.gpsimd.dma_start(out=we_sb, in_=we_d[:, :])
            wo_sb = const.tile([128, 4, DN], bf)
            nc.gpsimd.dma_start(out=wo_sb, in_=wo_d[:, :].rearrange("(c p) d -> p c d", p=128))
            bq_sb = const.tile([DH, H], f32)
            nc.gpsimd.dma_start(out=bq_sb, in_=bq_d[:].rearrange("(h d) -> d h", h=H))
            nodes_sb = const.tile([128, 4, DN], bf)
            nc.gpsimd.dma_start(out=nodes_sb, in_=nodes_d[:, :].rearrange("(t p) d -> p t d", p=128))
            nmy_sb = const.tile([NI, DN], bf)
            nc.gpsimd.dma_start(out=nmy_sb, in_=nmy_d[:, :])

            # ---- nodesT (128 dn, 512 n) via PE transposes ----
            nodesT = const.tile([DN, N], bf)
            for t in range(4):
                pt = ps_misc.tile([128, 128], bf, tag="m", name="pt")
                nc.tensor.matmul(out=pt, lhsT=nodes_sb[:, t, :], rhs=ident,
                                 start=True, stop=True, is_transpose=True)
                nc.vector.tensor_copy(out=nodesT[:, t * 128:(t + 1) * 128], in_=pt)
            # nodes_myT (128 dn, 64 i)
            nmyT = const.tile([DN, NI], bf)
            pt = ps_misc.tile([128, 128], bf, tag="m", name="pt2")
            nc.tensor.matmul(out=pt[:, 0:NI], lhsT=nmy_sb, rhs=ident[0:NI, 0:NI],
                             start=True, stop=True, is_transpose=True)
            nc.vector.tensor_copy(out=nmyT, in_=pt[:, 0:NI])
            # WeT (64 dh, 8, 64 de)
            weT = const.tile([DH, H, DE], bf)
            for h in range(H):
                pt = ps_misc.tile([128, 128], bf, tag="m", name="pt3")
                nc.tensor.matmul(out=pt[0:DH, 0:DE], lhsT=we_sb[:, h * DH:(h + 1) * DH],
                                 rhs=ident[0:DE, 0:DE], start=True, stop=True, is_transpose=True)
                nc.vector.tensor_copy(out=weT[:, h, :], in_=pt[0:DH, 0:DE])

            # ---- projections ----
            kT = const.tile([DH, H, N], bf)           # (dh, h, j)
            for h in range(H):
                pk = ps_misc.tile([DH, N], f32, tag="m", name="pk")
                nc.tensor.matmul(out=pk, lhsT=wk_sb[:, h * DH:(h + 1) * DH], rhs=nodesT,
                                 start=True, stop=True)
                nc.vector.tensor_copy(out=kT[:, h, :], in_=pk)
            qT = const.tile([DH, H, NI], bf)          # (dh, h, i) with bq added
            for h in range(H):
                pq = ps_misc.tile([DH, NI], f32, tag="m", name="pq")
                nc.tensor.matmul(out=pq, lhsT=wq_sb[:, h * DH:(h + 1) * DH], rhs=nmyT,
                                 start=True, stop=True)
                nc.vector.tensor_scalar_add(out=qT[:, h, :], in0=pq, scalar1=bq_sb[:, h:h + 1])
            v4 = const.tile([128, NJT, INNER], bf)    # (j, jt, (h dh))
            for jt in range(NJT):
                pv = ps_misc.tile([128, INNER], f32, tag="m", name="pv")
                nc.tensor.matmul(out=pv, lhsT=nodesT[:, jt * 128:(jt + 1) * 128], rhs=wv_sb,
                                 start=True, stop=True)
                nc.vector.tensor_copy(out=v4[:, jt, :], in_=pv)
            # u2: paired block-diag u (128 = 2x64 de, 32 pairs, 16 = 2x8 h)
            u2 = const.tile([128, NP, 16], bf)
            nc.gpsimd.memset(u2, 0.0)
            for h in range(H):
                pu = ps_misc.tile([DE, NI], f32, tag="m", name="pu")
                nc.tensor.matmul(out=pu, lhsT=weT[:, h, :], rhs=qT[:, h, :],
                                 start=True, stop=True)
                puv = pu.rearrange("d (p two) -> d p two", two=2)
                nc.vector.tensor_copy(out=u2[0:DE, :, h], in_=puv[:, :, 0])
                nc.vector.tensor_copy(out=u2[DE:128, :, 8 + h], in_=puv[:, :, 1])

            # ---- logits simT (j, i, h) per j-tile; exp ----
            expT = const.tile([128, NJT, NI * H], bf)
            for jt in range(NJT):
                simT = ps_simT.tile([128, NI, H], f32, tag="simT", name="simT")
                for h in range(H):
                    nc.tensor.matmul(out=simT[:, :, h],
                                     lhsT=kT[:, h, jt * 128:(jt + 1) * 128],
                                     rhs=qT[:, h, :],
                                     start=(h == 0), stop=False, skip_group_check=True)
                for ip in range(NP):
                    nc.tensor.matmul(out=simT[:, 2 * ip:2 * ip + 2, :],
                                     lhsT=eT_big[:, ip, jt * 128:(jt + 1) * 128],
                                     rhs=u2[:, ip, :],
                                     start=False, stop=(ip == NP - 1), skip_group_check=True)
                nc.scalar.activation(out=expT[:, jt, :], in_=simT[:, :, :],
                                     func=mybir.ActivationFunctionType.Exp, scale=SCALE)

            # ---- softmax denominators + attnT ----
            den = ps_misc.tile([128, NI * H], f32, tag="m", name="den")
            for jt in range(NJT):
                nc.tensor.matmul(out=den, lhsT=ones, rhs=expT[:, jt, :],
                                 start=(jt == 0), stop=(jt == NJT - 1), skip_group_check=True)
            recip = const.tile([128, NI * H], f32)
            nc.vector.reciprocal(out=recip, in_=den)
            recip_bf = const.tile([128, NI * H], bf)
            nc.vector.tensor_copy(out=recip_bf, in_=recip)
            attnT = const.tile([128, NJT, NI * H], bf)
            for jt in range(NJT):
                (nc.vector if jt % 2 == 0 else nc.gpsimd).tensor_mul(
                    out=attnT[:, jt, :], in0=expT[:, jt, :], in1=recip_bf)

            attv = attnT.rearrange("p t (i h) -> p t i h", h=H)

            # ---- out = attn @ v into one psum bank ----
            pout = ps_acc.tile([DH, H, NI], f32)
            for h in range(H):
                for jt in range(NJT):
                    nc.tensor.matmul(out=pout[:, h, :],
                                     lhsT=v4[:, jt, h * DH:(h + 1) * DH],
                                     rhs=attv[:, jt, :, h],
                                     start=(h == 0 and jt == 0), stop=False,
                                     skip_group_check=True)

            # ---- w[i] = attn[i] @ edges[i]  (pair-batched; off-diag garbage) ----
            w_sb = const.tile([DE, NI, H], bf)
            for ip in range(NP):
                pw = ps_w.tile([128, 16], f32, tag="pw", name="pw")
                for jt in range(NJT):
                    nc.tensor.matmul(out=pw,
                                     lhsT=en_t[jt][:, 2 * ip:2 * ip + 2, :],
                                     rhs=attv[:, jt, 2 * ip:2 * ip + 2, :],
                                     start=(jt == 0), stop=(jt == NJT - 1),
                                     skip_group_check=True)
                nc.vector.tensor_copy(out=w_sb[:, 2 * ip, :], in_=pw[0:DE, 0:8])
                nc.vector.tensor_copy(out=w_sb[:, 2 * ip + 1, :], in_=pw[DE:128, 8:16])

            # ---- out_e: (w @ We_h) accumulated into pout ----
            for h in range(H):
                nc.tensor.matmul(out=pout[:, h, :],
                                 lhsT=we_sb[:, h * DH:(h + 1) * DH],
                                 rhs=w_sb[:, :, h],
                                 start=False, stop=(h == H - 1),
                                 skip_group_check=True)

            # ---- final projection ----
            oiT = const.tile([128, 4, NI], bf)        # ((h dh) chunk, c, i)
            for h in range(H):
                nc.vector.tensor_copy(out=oiT[(h % 2) * DH:(h % 2) * DH + DH, h // 2, :],
                                      in_=pout[:, h, :])
            pfin = ps_misc.tile([DN, NI], f32, tag="m", name="pfin")
            for c in range(4):
                nc.tensor.matmul(out=pfin, lhsT=wo_sb[:, c, :], rhs=oiT[:, c, :],
                                 start=(c == 0), stop=(c == 3), skip_group_check=True)
            fin_sb = const.tile([DN, NI], bf)
            nc.vector.tensor_copy(out=fin_sb, in_=pfin)
            pft = ps_misc.tile([NI, DN], bf, tag="m", name="pft")
            nc.tensor.matmul(out=pft, lhsT=fin_sb, rhs=ident, start=True, stop=True,
                             is_transpose=True)
            out_sb = const.tile([NI, DN], f32)
            nc.vector.tensor_copy(out=out_sb, in_=pft)
            nc.gpsimd.dma_start(out=out_d[:, :], in_=out_sb)

    nc.finalize()
    return nc


def kernel(nodes, edges, mask, Wq, bq, Wk, bk, Wv, bv, We, be, Wo, bo):
    from concourse.bass_utils import run_bass_kernel_spmd

    nodes = np.asarray(nodes, np.float32)
    edges = np.asarray(edges, np.float32)
    mask = np.asarray(mask)
    Wq = np.asarray(Wq, np.float32); bq = np.asarray(bq, np.float32)
    Wk = np.asarray(Wk, np.float32)
    Wv = np.asarray(Wv, np.float32); bv = np.asarray(bv, np.float32)
    We = np.asarray(We, np.float32); be = np.asarray(be, np.float32)
    Wo = np.asarray(Wo, np.float32); bo = np.asarray(bo, np.float32)
    assert mask.all(), "kernel assumes an all-true mask (spec fill=ones)"

    if "nc" not in _CACHE:
        _CACHE["nc"] = _build()
    nc = _CACHE["nc"]

    n0 = nodes[0].astype(BF16)
    e_bf = edges[0].astype(BF16)
    wq_b = Wq.astype(BF16); wk_b = Wk.astype(BF16); wv_b = Wv.astype(BF16)
    we_b = We.astype(BF16); wo_b = Wo.astype(BF16)
    in_maps = []
    for c in range(NCORES):
        sl = e_bf[c * NI:(c + 1) * NI]
        in_maps.append({
            "nodes": n0,
            "nodes_my": n0[c * NI:(c + 1) * NI],
            "edges_nat": np.ascontiguousarray(sl.transpose(1, 0, 2)),
            "edges_T": np.ascontiguousarray(sl.transpose(0, 2, 1)),
            "Wq": wq_b, "Wk": wk_b, "Wv": wv_b, "We": we_b, "Wo": wo_b, "bq": bq,
        })

    trace = bool(os.environ.get("BASS_KERNEL_TRACE"))
    kw = {}
    if trace:
        _install_ntff_hook()
        import concourse.bass_utils as bu
        bu.upload_artifacts = lambda tmpdir: "local://skipped"
        kw = dict(trace=True, tmpdir=os.environ.get("BASS_KERNEL_TRACE_DIR") or None)
    res = run_bass_kernel_spmd(nc, in_maps, list(range(NCORES)), **kw)
    _CACHE["last_exec_ns"] = res.exec_time_ns

    out = np.concatenate([res.results[c]["out_my"] for c in range(NCORES)], axis=0)
    out = out + ((be + bv) @ Wo + bo)[None, :]
    return out.reshape(B, N, DN).astype(np.float32)


# revision 11
# speedup vs baseline: 4.0641x; 1.2274x over previous
"""Trainium2 Bass kernel for nn_Attention_35605278884484 (edge-augmented MHA).

B=1, N=512 nodes, H=8 heads, DH=64, DN=128 node feat, DE=64 edge feat.

Math (reference):
    q,k,v = nodes@W{q,k,v}+b ; e = edges@We+be (per head slices)
    sim[h,i,j] = scale * q[h,i] . (k[h,j] + e[h,i,j])
    attn = softmax_j(sim) ; out[h,i] = sum_j attn * (v[h,j] + e[h,i,j])
    final = concat_h(out) @ Wo + bo

Algebraic reductions used here (avoid materializing e, O(n^2 d_inner)):
    q.e[i,j]   = edges[i,j] . u[h,i],  u[h,i] = We_h @ q[h,i]
    (bk, be drop out of softmax: constant-in-j logit shifts)
    sum_j attn*e = (attn[i] @ edges[i]) @ We_h + be_h  (sum_j attn = 1)
    => be, bv fold into a host-side output bias (be+bv) @ Wo + bo

Sharding: sequence (i) sharded, 64 query rows per core; each core reads
only its (64, 512, 64) slice of edges (cast to bf16 on host) and
computes all 8 heads. Host concatenates the 8 (64, 128) results.

Device layout: "transposed" simT (j-partitions, (i,h) free):
- qk from kT/qT projections (strided psum writes per head),
- edge logit term: one block-diag matmul per i-pair
  (lhsT = PE-transposed edge pair (128=2x64de, 128j), rhs = zero-padded
  u-pair (128, 16)) accumulating into the same psum bank,
- softmax denominators via ones-matmul over j partitions,
- attn-edge output term w = attn @ edges also i-pair-batched (off-diag
  blocks are discarded garbage).
All PE operands are bf16 (fp32 matmul runs double-pass LOW_HIGH on
trn2); accumulations stay fp32 in PSUM.
"""
import os
import sys
import types
import contextlib

sys.path.insert(0, '/opt/trn_rl_repo')
sys.path.insert(0, '/root/.axon_site')

import numpy as np
import ml_dtypes

H, DH = 8, 64
B, N, DN, DE = 1, 512, 128, 64
INNER = H * DH
NCORES = 8
NI = N // NCORES          # 64 query rows per core
NP = NI // 2              # 32 i-pairs per core
NJT = N // 128            # 4 j tiles
SCALE = float(DH ** -0.5)
BF16 = ml_dtypes.bfloat16

_CACHE = {}


def _install_ntff_hook():
    """antenv.axon_hooks is absent in this image; synthesize it so
    run_bass_kernel_spmd(trace=True) can profile via libaxon."""
    if "antenv.axon_hooks" in sys.modules:
        return
    try:
        from trn_agent_boot.trn_boot import _ntff_profile_via_ctypes
        hook = _ntff_profile_via_ctypes('/opt/axon/libaxon_pjrt.so')
    except Exception:
        hook = None
    mod = types.ModuleType("antenv.axon_hooks")
    mod.get_axon_ntff_profile_hook = lambda: hook
    mod.set_axon_ntff_profile_hook = lambda h: None
    sys.modules["antenv.axon_hooks"] = mod


def _build():
    import concourse.mybir as mybir
    from concourse import bacc
    from concourse.tile import TileContext
    from concourse.masks import make_identity

    f32 = mybir.dt.float32
    bf = mybir.dt.bfloat16
    nc = bacc.Bacc(None, target_bir_lowering=False)

    nodes_d = nc.declare_dram_parameter("nodes", [N, DN], bf, isOutput=False)
    nmy_d = nc.declare_dram_parameter("nodes_my", [NI, DN], bf, isOutput=False)
    enat_d = nc.declare_dram_parameter("edges_nat", [N, NI, DE], bf, isOutput=False)
    etr_d = nc.declare_dram_parameter("edges_T", [128, NP, N], bf, isOutput=False)
    wq_d = nc.declare_dram_parameter("Wq", [DN, INNER], bf, isOutput=False)
    wk_d = nc.declare_dram_parameter("Wk", [DN, INNER], bf, isOutput=False)
    wv_d = nc.declare_dram_parameter("Wv", [DN, INNER], bf, isOutput=False)
    we_d = nc.declare_dram_parameter("We", [DE, INNER], bf, isOutput=False)
    wo_d = nc.declare_dram_parameter("Wo", [INNER, DN], bf, isOutput=False)
    bq_d = nc.declare_dram_parameter("bq", [INNER], f32, isOutput=False)
    out_d = nc.declare_dram_parameter("out_my", [NI, DN], f32, isOutput=True)

    with TileContext(nc) as tc:
        with contextlib.ExitStack() as ctx:
            const = ctx.enter_context(tc.tile_pool(name="const", bufs=1))
            en_pool = ctx.enter_context(tc.tile_pool(name="edges", bufs=1))
            # PSUM budget is 8 banks; every tile below pads to 1 bank.
            ps_simT = ctx.enter_context(tc.tile_pool(name="ps_simT", bufs=3, space="PSUM"))
            ps_misc = ctx.enter_context(tc.tile_pool(name="ps_misc", bufs=2, space="PSUM"))
            ps_acc = ctx.enter_context(tc.tile_pool(name="ps_acc", bufs=1, space="PSUM"))
            ps_w = ctx.enter_context(tc.tile_pool(name="ps_w", bufs=2, space="PSUM"))

            # ---- constants / weights ----
            ident = const.tile([128, 128], bf)
            make_identity(nc, ident)
            ones = const.tile([128, 128], bf)
            nc.vector.memset(ones, 1.0)

            wq_sb = const.tile([DN, INNER], bf)
            nc.sync.dma_start(out=wq_sb, in_=wq_d[:, :])
            wk_sb = const.tile([DN, INNER], bf)
            nc.scalar.dma_start(out=wk_sb, in_=wk_d[:, :])
            wv_sb = const.tile([DN, INNER], bf)
            nc.sync.dma_start(out=wv_sb, in_=wv_d[:, :])
            we_sb = const.tile([DE, INNER], bf)
            nc.scalar.dma_start(out=we_sb, in_=we_d[:, :])
            wo_sb = const.tile([128, 4, DN], bf)
            nc.sync.dma_start(out=wo_sb, in_=wo_d[:, :].rearrange("(c p) d -> p c d", p=128))
            bq_sb = const.tile([DH, H], f32)
            nc.gpsimd.dma_start(out=bq_sb, in_=bq_d[:].rearrange("(h d) -> d h", h=H))
            nodes_sb = const.tile([128, 4, DN], bf)
            nc.scalar.dma_start(out=nodes_sb, in_=nodes_d[:, :].rearrange("(t p) d -> p t d", p=128))
            nmy_sb = const.tile([NI, DN], bf)
            nc.sync.dma_start(out=nmy_sb, in_=nmy_d[:, :])

            # ---- small weight/node loads first (fast), then edge streams ----
            # eT_big: (128 = (i-parity, de), pair, j) -- lhsT tiles for the
            # block-diag logit matmuls. Fully contiguous (1KB runs).
            eT_big = en_pool.tile([128, NP, N], bf, tag="eTb", name="eT_big")
            for g in range(4):
                eng = nc.scalar if g % 2 == 0 else nc.sync
                eng.dma_start(out=eT_big[:, g * 8:(g + 1) * 8, :],
                              in_=etr_d[:, g * 8:(g + 1) * 8, :])
            # e_nat: (j, i, de) per j-tile for the w contraction. Contiguous.
            en_t = [en_pool.tile([128, NI, DE], bf, tag=f"en{jt}", name=f"en{jt}")
                    for jt in range(NJT)]
            for jt in range(NJT):
                eng = nc.sync if jt % 2 == 0 else nc.scalar
                eng.dma_start(out=en_t[jt],
                              in_=enat_d[jt * 128:(jt + 1) * 128, :, :])

            # ---- nodesT (128 dn, 512 n) via PE transposes ----
            nodesT = const.tile([DN, N], bf)
            for t in range(4):
                pt = ps_misc.tile([128, 128], bf, tag="m", name="pt")
                nc.tensor.matmul(out=pt, lhsT=nodes_sb[:, t, :], rhs=ident,
                                 start=True, stop=True, is_transpose=True)
                nc.vector.tensor_copy(out=nodesT[:, t * 128:(t + 1) * 128], in_=pt)
            # nodes_myT (128 dn, 64 i)
            nmyT = const.tile([DN, NI], bf)
            pt = ps_misc.tile([128, 128], bf, tag="m", name="pt2")
            nc.tensor.matmul(out=pt[:, 0:NI], lhsT=nmy_sb, rhs=ident[0:NI, 0:NI],
                             start=True, stop=True, is_transpose=True)
            nc.vector.tensor_copy(out=nmyT, in_=pt[:, 0:NI])
            # WeT (64 dh, 8, 64 de)
            weT = const.tile([DH, H, DE], bf)
            for h in range(H):
                pt = ps_misc.tile([128, 128], bf, tag="m", name="pt3")
                nc.tensor.matmul(out=pt[0:DH, 0:DE], lhsT=we_sb[:, h * DH:(h + 1) * DH],
                                 rhs=ident[0:DE, 0:DE], start=True, stop=True, is_transpose=True)
                nc.vector.tensor_copy(out=weT[:, h, :], in_=pt[0:DH, 0:DE])

            # ---- projections ----
            kT = const.tile([DH, H, N], bf)           # (dh, h, j)
            for h in range(H):
                pk = ps_misc.tile([DH, N], f32, tag="m", name="pk")
                nc.tensor.matmul(out=pk, lhsT=wk_sb[:, h * DH:(h + 1) * DH], rhs=nodesT,
                                 start=True, stop=True)
                nc.vector.tensor_copy(out=kT[:, h, :], in_=pk)
            qT = const.tile([DH, H, NI], bf)          # (dh, h, i) with bq added
            for h in range(H):
                pq = ps_misc.tile([DH, NI], f32, tag="m", name="pq")
                nc.tensor.matmul(out=pq, lhsT=wq_sb[:, h * DH:(h + 1) * DH], rhs=nmyT,
                                 start=True, stop=True)
                nc.vector.tensor_scalar_add(out=qT[:, h, :], in0=pq, scalar1=bq_sb[:, h:h + 1])
            v4 = const.tile([128, NJT, INNER], bf)    # (j, jt, (h dh))
            for jt in range(NJT):
                pv = ps_misc.tile([128, INNER], f32, tag="m", name="pv")
                nc.tensor.matmul(out=pv, lhsT=nodesT[:, jt * 128:(jt + 1) * 128], rhs=wv_sb,
                                 start=True, stop=True)
                nc.vector.tensor_copy(out=v4[:, jt, :], in_=pv)
            # u2: paired block-diag u (128 = 2x64 de, 32 pairs, 16 = 2x8 h)
            u2 = const.tile([128, NP, 16], bf)
            nc.gpsimd.memset(u2, 0.0)
            for h in range(H):
                pu = ps_misc.tile([DE, NI], f32, tag="m", name="pu")
                nc.tensor.matmul(out=pu, lhsT=weT[:, h, :], rhs=qT[:, h, :],
                                 start=True, stop=True)
                puv = pu.rearrange("d (p two) -> d p two", two=2)
                nc.vector.tensor_copy(out=u2[0:DE, :, h], in_=puv[:, :, 0])
                nc.vector.tensor_copy(out=u2[DE:128, :, 8 + h], in_=puv[:, :, 1])

            # ---- logits simT (j, i, h) per j-tile; exp ----
            expT = const.tile([128, NJT, NI * H], bf)
            for jt in range(NJT):
                simT = ps_simT.tile([128, NI, H], f32, tag="simT", name="simT")
                for h in range(H):
                    nc.tensor.matmul(out=simT[:, :, h],
                                     lhsT=kT[:, h, jt * 128:(jt + 1) * 128],
                                     rhs=qT[:, h, :],
                                     start=(h == 0), stop=False, skip_group_check=True)
                for ip in range(NP):
                    nc.tensor.matmul(out=simT[:, 2 * ip:2 * ip + 2, :],
                                     lhsT=eT_big[:, ip, jt * 128:(jt + 1) * 128],
                                     rhs=u2[:, ip, :],
                                     start=False, stop=(ip == NP - 1), skip_group_check=True)
                nc.scalar.activation(out=expT[:, jt, :], in_=simT[:, :, :],
                                     func=mybir.ActivationFunctionType.Exp, scale=SCALE)

            # ---- softmax denominators + attnT ----
            den = ps_misc.tile([128, NI * H], f32, tag="m", name="den")
            for jt in range(NJT):
                nc.tensor.matmul(out=den, lhsT=ones, rhs=expT[:, jt, :],
                                 start=(jt == 0), stop=(jt == NJT - 1), skip_group_check=True)
            recip = const.tile([128, NI * H], f32)
            nc.vector.reciprocal(out=recip, in_=den)
            recip_bf = const.tile([128, NI * H], bf)
            nc.vector.tensor_copy(out=recip_bf, in_=recip)
            attnT = const.tile([128, NJT, NI * H], bf)
            for jt in range(NJT):
                (nc.vector if jt % 2 == 0 else nc.gpsimd).tensor_mul(
                    out=attnT[:, jt, :], in0=expT[:, jt, :], in1=recip_bf)

            attv = attnT.rearrange("p t (i h) -> p t i h", h=H)

            # ---- out = attn @ v into one psum bank ----
            pout = ps_acc.tile([DH, H, NI], f32)
            for h in range(H):
                for jt in range(NJT):
                    nc.tensor.matmul(out=pout[:, h, :],
                                     lhsT=v4[:, jt, h * DH:(h + 1) * DH],
                                     rhs=attv[:, jt, :, h],
                                     start=(h == 0 and jt == 0), stop=False,
                                     skip_group_check=True)

            # ---- w[i] = attn[i] @ edges[i]  (pair-batched; off-diag garbage) ----
            w_sb = const.tile([DE, NI, H], bf)
            wv2 = w_sb.rearrange("d (i2 two) h -> d i2 two h", two=2)
            for g in range(8):
                pw = ps_w.tile([128, 4, 16], f32, tag="pw", name="pw")
                for pi in range(4):
                    ip = g * 4 + pi
                    for jt in range(NJT):
                        nc.tensor.matmul(out=pw[:, pi, :],
                                         lhsT=en_t[jt][:, 2 * ip:2 * ip + 2, :],
                                         rhs=attv[:, jt, 2 * ip:2 * ip + 2, :],
                                         start=(pi == 0 and jt == 0),
                                         stop=(pi == 3 and jt == NJT - 1),
                                         skip_group_check=True)
                nc.vector.tensor_copy(out=wv2[:, 4 * g:4 * g + 4, 0, :],
                                      in_=pw[0:DE, :, 0:8])
                nc.vector.tensor_copy(out=wv2[:, 4 * g:4 * g + 4, 1, :],
                                      in_=pw[DE:128, :, 8:16])

            # ---- out_e: (w @ We_h) accumulated into pout ----
            for h in range(H):
                nc.tensor.matmul(out=pout[:, h, :],
                                 lhsT=we_sb[:, h * DH:(h + 1) * DH],
                                 rhs=w_sb[:, :, h],
                                 start=False, stop=(h == H - 1),
                                 skip_group_check=True)

            # ---- final projection ----
            oiT = const.tile([128, 4, NI], bf)        # ((h dh) chunk, c, i)
            for h in range(H):
                nc.vector.tensor_copy(out=oiT[(h % 2) * DH:(h % 2) * DH + DH, h // 2, :],
                                      in_=pout[:, h, :])
            pfin = ps_misc.tile([DN, NI], f32, tag="m", name="pfin")
            for c in range(4):
                nc.tensor.matmul(out=pfin, lhsT=wo_sb[:, c, :], rhs=oiT[:, c, :],
                                 start=(c == 0), stop=(c == 3), skip_group_check=True)
            fin_sb = const.tile([DN, NI], bf)
            nc.vector.tensor_copy(out=fin_sb, in_=pfin)
            pft = ps_misc.tile([NI, DN], bf, tag="m", name="pft")
            nc.tensor.matmul(out=pft, lhsT=fin_sb, rhs=ident, start=True, stop=True,
                             is_transpose=True)
            out_sb = const.tile([NI, DN], f32)
            nc.vector.tensor_copy(out=out_sb, in_=pft)
            nc.gpsimd.dma_start(out=out_d[:, :], in_=out_sb)

    nc.finalize()
    return nc


def kernel(nodes, edges, mask, Wq, bq, Wk, bk, Wv, bv, We, be, Wo, bo):
    from concourse.bass_utils import run_bass_kernel_spmd

    nodes = np.asarray(nodes, np.float32)
    edges = np.asarray(edges, np.float32)
    mask = np.asarray(mask)
    Wq = np.asarray(Wq, np.float32); bq = np.asarray(bq, np.float32)
    Wk = np.asarray(Wk, np.float32)
    Wv = np.asarray(Wv, np.float32); bv = np.asarray(bv, np.float32)
    We = np.asarray(We, np.float32); be = np.asarray(be, np.float32)
    Wo = np.asarray(Wo, np.float32); bo = np.asarray(bo, np.float32)
    assert mask.all(), "kernel assumes an all-true mask (spec fill=ones)"

    if "nc" not in _CACHE:
        _CACHE["nc"] = _build()
    nc = _CACHE["nc"]

    n0 = nodes[0].astype(BF16)
    e_bf = edges[0].astype(BF16)
    wq_b = Wq.astype(BF16); wk_b = Wk.astype(BF16); wv_b = Wv.astype(BF16)
    we_b = We.astype(BF16); wo_b = Wo.astype(BF16)
    in_maps = []
    for c in range(NCORES):
        sl = e_bf[c * NI:(c + 1) * NI]
        in_maps.append({
            "nodes": n0,
            "nodes_my": n0[c * NI:(c + 1) * NI],
            "edges_nat": np.ascontiguousarray(sl.transpose(1, 0, 2)),
            "edges_T": np.ascontiguousarray(
                sl.transpose(0, 2, 1).reshape(NP, 2, DE, N)
                .transpose(1, 2, 0, 3).reshape(128, NP, N)),
            "Wq": wq_b, "Wk": wk_b, "Wv": wv_b, "We": we_b, "Wo": wo_b, "bq": bq,
        })

    trace = bool(os.environ.get("BASS_KERNEL_TRACE"))
    kw = {}
    if trace:
        _install_ntff_hook()
        import concourse.bass_utils as bu
        bu.upload_artifacts = lambda tmpdir: "local://skipped"
        kw = dict(trace=True, tmpdir=os.environ.get("BASS_KERNEL_TRACE_DIR") or None)
    res = run_bass_kernel_spmd(nc, in_maps, list(range(NCORES)), **kw)
    _CACHE["last_exec_ns"] = res.exec_time_ns

    out = np.concatenate([res.results[c]["out_my"] for c in range(NCORES)], axis=0)
    out = out + ((be + bv) @ Wo + bo)[None, :]
    return out.reshape(B, N, DN).astype(np.float32)


# revision 12
# speedup vs baseline: 4.5062x; 1.1088x over previous
"""Trainium2 Bass kernel for nn_Attention_35605278884484 (edge-augmented MHA).

B=1, N=512 nodes, H=8 heads, DH=64, DN=128 node feat, DE=64 edge feat.

Math (reference):
    q,k,v = nodes@W{q,k,v}+b ; e = edges@We+be (per head slices)
    sim[h,i,j] = scale * q[h,i] . (k[h,j] + e[h,i,j])
    attn = softmax_j(sim) ; out[h,i] = sum_j attn * (v[h,j] + e[h,i,j])
    final = concat_h(out) @ Wo + bo

Algebraic reductions (avoid materializing e, O(n^2 d_inner)):
    q.e[i,j]   = edges[i,j] . u[h,i],  u[h,i] = We_h @ q[h,i]
    (bk, be drop out of softmax: constant-in-j logit shifts)
    sum_j attn*e = (attn[i] @ edges[i]) @ We_h + be_h  (sum_j attn = 1)
    => be, bv fold into a host-side output bias (be+bv) @ Wo + bo

Sharding: sequence (i) sharded, 64 query rows per core; each core reads
only its slice of edges (bf16, two host-prepared layouts) and computes
all 8 heads. Host concatenates the 8 per-core results.

Device pipeline (all PE operands bf16; fp32 accumulation in PSUM):
  simT (j-partitions, (i,h) free) per j-tile psum bank:
    qk via strided psum writes per head (kT/qT from host-pretransposed
    projections), edge term via one block-diag matmul per i-pair
    (lhsT = edge-pair slice of the host-transposed eT image,
     rhs = zero-padded u-pair (128, 16)).
  exp on ACT (column chunks), denominators via ones-matmul over j,
  attn = exp * recip (DVE/GpSimd), attn@v and the pair-batched
  w = attn@edges -> (w @ We_h) accumulate into one output psum bank,
  final Wo projection; output written (dn, i), host transposes back.
"""
import os
import sys
import types
import contextlib

sys.path.insert(0, '/opt/trn_rl_repo')
sys.path.insert(0, '/root/.axon_site')

import numpy as np
import ml_dtypes

H, DH = 8, 64
B, N, DN, DE = 1, 512, 128, 64
INNER = H * DH
NCORES = 8
NI = N // NCORES          # 64 query rows per core
NP = NI // 2              # 32 i-pairs per core
NJT = N // 128            # 4 j tiles
SCALE = float(DH ** -0.5)
BF16 = ml_dtypes.bfloat16

# wpack column offsets (all bf16, 128 rows)
C_NT = 0        # nodesT (128, 512)
C_WQ = 512      # Wq (128, 512)
C_WK = 1024     # Wk
C_WV = 1536     # Wv
C_WO = 2048     # Wo chunks (128, 4*128)
C_WE = 2560     # We (rows 0:64, 512)
C_WET = 3072    # WeT (rows 0:64, (h, de) 512)
C_NMY = 3584    # nodes_myT (128, 64)
WPCOLS = 3648

_CACHE = {}


def _install_ntff_hook():
    """antenv.axon_hooks is absent in this image; synthesize it so
    run_bass_kernel_spmd(trace=True) can profile via libaxon."""
    if "antenv.axon_hooks" in sys.modules:
        return
    try:
        from trn_agent_boot.trn_boot import _ntff_profile_via_ctypes
        hook = _ntff_profile_via_ctypes('/opt/axon/libaxon_pjrt.so')
    except Exception:
        hook = None
    mod = types.ModuleType("antenv.axon_hooks")
    mod.get_axon_ntff_profile_hook = lambda: hook
    mod.set_axon_ntff_profile_hook = lambda h: None
    sys.modules["antenv.axon_hooks"] = mod


def _build():
    import concourse.mybir as mybir
    from concourse import bacc
    from concourse.tile import TileContext

    f32 = mybir.dt.float32
    bf = mybir.dt.bfloat16
    nc = bacc.Bacc(None, target_bir_lowering=False)

    wp_d = nc.declare_dram_parameter("wpack", [128, WPCOLS], bf, isOutput=False)
    bq_d = nc.declare_dram_parameter("bq2", [DH, H], f32, isOutput=False)
    enat_d = nc.declare_dram_parameter("edges_nat", [N, NI, DE], bf, isOutput=False)
    etr_d = nc.declare_dram_parameter("edges_T", [128, NP, N], bf, isOutput=False)
    out_d = nc.declare_dram_parameter("out_my", [DN, NI], f32, isOutput=True)

    with TileContext(nc) as tc:
        with contextlib.ExitStack() as ctx:
            const = ctx.enter_context(tc.tile_pool(name="const", bufs=1))
            en_pool = ctx.enter_context(tc.tile_pool(name="edges", bufs=1))
            # PSUM budget is 8 banks; every tile below pads to 1 bank.
            ps_simT = ctx.enter_context(tc.tile_pool(name="ps_simT", bufs=3, space="PSUM"))
            ps_misc = ctx.enter_context(tc.tile_pool(name="ps_misc", bufs=2, space="PSUM"))
            ps_acc = ctx.enter_context(tc.tile_pool(name="ps_acc", bufs=1, space="PSUM"))
            ps_w = ctx.enter_context(tc.tile_pool(name="ps_w", bufs=2, space="PSUM"))

            # ---- one packed weight DMA + bq, then the edge streams ----
            wp = const.tile([128, WPCOLS], bf)
            nc.sync.dma_start(out=wp, in_=wp_d[:, :])
            bq_sb = const.tile([DH, H], f32)
            nc.gpsimd.dma_start(out=bq_sb, in_=bq_d[:, :])

            eT_big = en_pool.tile([128, NP, N], bf, tag="eTb", name="eT_big")
            for g in range(4):
                eng = nc.scalar if g % 2 == 0 else nc.sync
                eng.dma_start(out=eT_big[:, g * 8:(g + 1) * 8, :],
                              in_=etr_d[:, g * 8:(g + 1) * 8, :])
            en_t = [en_pool.tile([128, NI, DE], bf, tag=f"en{jt}", name=f"en{jt}")
                    for jt in range(NJT)]
            for jt in range(NJT):
                eng = nc.sync if jt % 2 == 0 else nc.scalar
                eng.dma_start(out=en_t[jt],
                              in_=enat_d[jt * 128:(jt + 1) * 128, :, :])

            nodesT = wp[:, C_NT:C_NT + N]
            nmyT = wp[:, C_NMY:C_NMY + NI]
            ones = const.tile([128, 128], bf)
            nc.vector.memset(ones, 1.0)

            # ---- projections (qT/u2 chain first: it gates the sim phase) ----
            qT = const.tile([DH, H, NI], bf)          # (dh, h, i) with bq added
            for h in range(H):
                pq = ps_misc.tile([DH, NI], f32, tag="m", name="pq")
                nc.tensor.matmul(out=pq, lhsT=wp[:, C_WQ + h * DH:C_WQ + (h + 1) * DH],
                                 rhs=nmyT, start=True, stop=True)
                nc.vector.tensor_scalar_add(out=qT[:, h, :], in0=pq, scalar1=bq_sb[:, h:h + 1])
            # u2: paired block-diag u (128 = 2x64 de, 32 pairs, 16 = 2x8 h)
            u2 = const.tile([128, NP, 16], bf)
            nc.gpsimd.memset(u2, 0.0)
            for h in range(H):
                pu = ps_misc.tile([DE, NI], f32, tag="m", name="pu")
                nc.tensor.matmul(out=pu, lhsT=wp[0:DE, C_WET + h * DH:C_WET + (h + 1) * DH],
                                 rhs=qT[:, h, :], start=True, stop=True)
                puv = pu.rearrange("d (p two) -> d p two", two=2)
                nc.vector.tensor_copy(out=u2[0:DE, :, h], in_=puv[:, :, 0])
                nc.vector.tensor_copy(out=u2[DE:128, :, 8 + h], in_=puv[:, :, 1])
            kT = const.tile([DH, H, N], bf)           # (dh, h, j)
            for h in range(H):
                pk = ps_misc.tile([DH, N], f32, tag="m", name="pk")
                nc.tensor.matmul(out=pk, lhsT=wp[:, C_WK + h * DH:C_WK + (h + 1) * DH],
                                 rhs=nodesT, start=True, stop=True)
                nc.vector.tensor_copy(out=kT[:, h, :], in_=pk)
            v4 = const.tile([128, NJT, INNER], bf)    # (j, jt, (h dh))
            for jt in range(NJT):
                pv = ps_misc.tile([128, INNER], f32, tag="m", name="pv")
                nc.tensor.matmul(out=pv, lhsT=nodesT[:, jt * 128:(jt + 1) * 128],
                                 rhs=wp[:, C_WV:C_WV + INNER], start=True, stop=True)
                nc.vector.tensor_copy(out=v4[:, jt, :], in_=pv)

            # ---- logits simT (j, i, h) per j-tile; exp in column chunks ----
            expT = const.tile([128, NJT, NI * H], bf)
            for jt in range(NJT):
                simT = ps_simT.tile([128, NI, H], f32, tag="simT", name="simT")
                for h in range(H):
                    nc.tensor.matmul(out=simT[:, :, h],
                                     lhsT=kT[:, h, jt * 128:(jt + 1) * 128],
                                     rhs=qT[:, h, :],
                                     start=(h == 0), stop=False, skip_group_check=True)
                for ip in range(NP):
                    nc.tensor.matmul(out=simT[:, 2 * ip:2 * ip + 2, :],
                                     lhsT=eT_big[:, ip, jt * 128:(jt + 1) * 128],
                                     rhs=u2[:, ip, :],
                                     start=False, stop=(ip == NP - 1), skip_group_check=True)
                for c in range(4):
                    nc.scalar.activation(out=expT[:, jt, c * 128:(c + 1) * 128],
                                         in_=simT[:, 16 * c:16 * (c + 1), :],
                                         func=mybir.ActivationFunctionType.Exp, scale=SCALE)

            # ---- chunked softmax denominators + attnT ----
            recip = const.tile([128, NI * H], f32)
            recip_bf = const.tile([128, NI * H], bf)
            attnT = const.tile([128, NJT, NI * H], bf)
            for c in range(4):
                cs = slice(c * 128, (c + 1) * 128)
                den = ps_misc.tile([128, 128], f32, tag="m", name=f"den{c}")
                for jt in range(NJT):
                    nc.tensor.matmul(out=den, lhsT=ones, rhs=expT[:, jt, cs],
                                     start=(jt == 0), stop=(jt == NJT - 1),
                                     skip_group_check=True)
                nc.vector.reciprocal(out=recip[:, cs], in_=den)
                nc.vector.tensor_copy(out=recip_bf[:, cs], in_=recip[:, cs])
                for jt in range(NJT):
                    (nc.vector if jt % 2 == 0 else nc.gpsimd).tensor_mul(
                        out=attnT[:, jt, cs], in0=expT[:, jt, cs], in1=recip_bf[:, cs])

            attv = attnT.rearrange("p t (i h) -> p t i h", h=H)

            # ---- w[i] = attn[i] @ edges[i] (pair-batched; off-diag garbage) ----
            # chunk order so w starts as soon as its attnT columns are ready
            w_sb = const.tile([DE, NI, H], bf)
            wv2 = w_sb.rearrange("d (i2 two) h -> d i2 two h", two=2)
            for g in range(8):
                pw = ps_w.tile([128, 4, 16], f32, tag="pw", name="pw")
                for pi in range(4):
                    ip = g * 4 + pi
                    for jt in range(NJT):
                        nc.tensor.matmul(out=pw[:, pi, :],
                                         lhsT=en_t[jt][:, 2 * ip:2 * ip + 2, :],
                                         rhs=attv[:, jt, 2 * ip:2 * ip + 2, :],
                                         start=(pi == 0 and jt == 0),
                                         stop=(pi == 3 and jt == NJT - 1),
                                         skip_group_check=True)
                nc.vector.tensor_copy(out=wv2[:, 4 * g:4 * g + 4, 0, :],
                                      in_=pw[0:DE, :, 0:8])
                nc.vector.tensor_copy(out=wv2[:, 4 * g:4 * g + 4, 1, :],
                                      in_=pw[DE:128, :, 8:16])

            # ---- attn @ v and the w @ We_h term, one output psum bank ----
            pout = ps_acc.tile([DH, H, NI], f32)
            for h in range(H):
                for jt in range(NJT):
                    nc.tensor.matmul(out=pout[:, h, :],
                                     lhsT=v4[:, jt, h * DH:(h + 1) * DH],
                                     rhs=attv[:, jt, :, h],
                                     start=(h == 0 and jt == 0), stop=False,
                                     skip_group_check=True)
            for h in range(H):
                nc.tensor.matmul(out=pout[:, h, :],
                                 lhsT=wp[0:DE, C_WE + h * DH:C_WE + (h + 1) * DH],
                                 rhs=w_sb[:, :, h],
                                 start=False, stop=(h == H - 1),
                                 skip_group_check=True)

            # ---- final projection; output stays transposed (dn, i) ----
            oiT = const.tile([128, 4, NI], bf)        # ((h dh) chunk, c, i)
            for h in range(H):
                nc.vector.tensor_copy(out=oiT[(h % 2) * DH:(h % 2) * DH + DH, h // 2, :],
                                      in_=pout[:, h, :])
            pfin = ps_misc.tile([DN, NI], f32, tag="m", name="pfin")
            for c in range(4):
                nc.tensor.matmul(out=pfin, lhsT=wp[:, C_WO + c * 128:C_WO + (c + 1) * 128],
                                 rhs=oiT[:, c, :],
                                 start=(c == 0), stop=(c == 3), skip_group_check=True)
            fin_sb = const.tile([DN, NI], f32)
            nc.vector.tensor_copy(out=fin_sb, in_=pfin)
            nc.gpsimd.dma_start(out=out_d[:, :], in_=fin_sb)

    nc.finalize()
    return nc


def kernel(nodes, edges, mask, Wq, bq, Wk, bk, Wv, bv, We, be, Wo, bo):
    from concourse.bass_utils import run_bass_kernel_spmd

    nodes = np.asarray(nodes, np.float32)
    edges = np.asarray(edges, np.float32)
    mask = np.asarray(mask)
    Wq = np.asarray(Wq, np.float32); bq = np.asarray(bq, np.float32)
    Wk = np.asarray(Wk, np.float32)
    Wv = np.asarray(Wv, np.float32); bv = np.asarray(bv, np.float32)
    We = np.asarray(We, np.float32); be = np.asarray(be, np.float32)
    Wo = np.asarray(Wo, np.float32); bo = np.asarray(bo, np.float32)
    assert mask.all(), "kernel assumes an all-true mask (spec fill=ones)"

    if "nc" not in _CACHE:
        _CACHE["nc"] = _build()
    nc = _CACHE["nc"]

    n0 = nodes[0].astype(BF16)
    e_bf = edges[0].astype(BF16)

    wp_base = np.zeros((128, WPCOLS), BF16)
    wp_base[:, C_NT:C_NT + N] = n0.T
    wp_base[:, C_WQ:C_WQ + INNER] = Wq.astype(BF16)
    wp_base[:, C_WK:C_WK + INNER] = Wk.astype(BF16)
    wp_base[:, C_WV:C_WV + INNER] = Wv.astype(BF16)
    wp_base[:, C_WO:C_WO + INNER] = (
        Wo.reshape(4, 128, DN).transpose(1, 0, 2).reshape(128, 4 * DN).astype(BF16))
    wp_base[0:DE, C_WE:C_WE + INNER] = We.astype(BF16)
    wp_base[0:DE, C_WET:C_WET + INNER] = (
        We.reshape(DE, H, DH).transpose(2, 1, 0).reshape(DH, H * DE).astype(BF16))
    bq2 = np.ascontiguousarray(bq.reshape(H, DH).T)

    in_maps = []
    for c in range(NCORES):
        sl = e_bf[c * NI:(c + 1) * NI]
        wp = wp_base.copy()
        wp[:, C_NMY:C_NMY + NI] = n0[c * NI:(c + 1) * NI].T
        in_maps.append({
            "wpack": wp,
            "bq2": bq2,
            "edges_nat": np.ascontiguousarray(sl.transpose(1, 0, 2)),
            "edges_T": np.ascontiguousarray(
                sl.transpose(0, 2, 1).reshape(NP, 2, DE, N)
                .transpose(1, 2, 0, 3).reshape(128, NP, N)),
        })

    trace = bool(os.environ.get("BASS_KERNEL_TRACE"))
    kw = {}
    if trace:
        _install_ntff_hook()
        import concourse.bass_utils as bu
        bu.upload_artifacts = lambda tmpdir: "local://skipped"
        kw = dict(trace=True, tmpdir=os.environ.get("BASS_KERNEL_TRACE_DIR") or None)
    res = run_bass_kernel_spmd(nc, in_maps, list(range(NCORES)), **kw)
    _CACHE["last_exec_ns"] = res.exec_time_ns

    out = np.concatenate([res.results[c]["out_my"].T for c in range(NCORES)], axis=0)
    out = out + ((be + bv) @ Wo + bo)[None, :]
    return out.reshape(B, N, DN).astype(np.float32)


# revision 13
# speedup vs baseline: 4.5079x; 1.0004x over previous
"""Trainium2 Bass kernel for nn_Attention_35605278884484 (edge-augmented MHA).

B=1, N=512 nodes, H=8 heads, DH=64, DN=128 node feat, DE=64 edge feat.

Math (reference):
    q,k,v = nodes@W{q,k,v}+b ; e = edges@We+be (per head slices)
    sim[h,i,j] = scale * q[h,i] . (k[h,j] + e[h,i,j])
    attn = softmax_j(sim) ; out[h,i] = sum_j attn * (v[h,j] + e[h,i,j])
    final = concat_h(out) @ Wo + bo

Algebraic reductions (avoid materializing e, O(n^2 d_inner)):
    q.e[i,j]   = edges[i,j] . u[h,i],  u[h,i] = We_h @ q[h,i]
    (bk, be drop out of softmax: constant-in-j logit shifts)
    sum_j attn*e = (attn[i] @ edges[i]) @ We_h + be_h  (sum_j attn = 1)
    => be, bv fold into a host-side output bias (be+bv) @ Wo + bo

Sharding: sequence (i) sharded, 64 query rows per core; each core reads
only its slice of edges (bf16, two host-prepared layouts) and computes
all 8 heads. Host concatenates the 8 per-core results.

Device pipeline (all PE operands bf16; fp32 accumulation in PSUM):
  simT (j-partitions, (i,h) free) per j-tile psum bank:
    qk via strided psum writes per head (kT/qT from host-pretransposed
    projections), edge term via one block-diag matmul per i-pair
    (lhsT = edge-pair slice of the host-transposed eT image,
     rhs = zero-padded u-pair (128, 16)).
  exp on ACT (column chunks), denominators via ones-matmul over j,
  attn = exp * recip (DVE/GpSimd), attn@v and the pair-batched
  w = attn@edges -> (w @ We_h) accumulate into one output psum bank,
  final Wo projection; output written (dn, i), host transposes back.
"""
import os
import sys
import types
import contextlib

sys.path.insert(0, '/opt/trn_rl_repo')
sys.path.insert(0, '/root/.axon_site')

import numpy as np
import ml_dtypes

H, DH = 8, 64
B, N, DN, DE = 1, 512, 128, 64
INNER = H * DH
NCORES = 8
NI = N // NCORES          # 64 query rows per core
NP = NI // 2              # 32 i-pairs per core
NJT = N // 128            # 4 j tiles
SCALE = float(DH ** -0.5)
BF16 = ml_dtypes.bfloat16

# wpack column offsets (all bf16, 128 rows)
C_NT = 0        # nodesT (128, 512)
C_WQ = 512      # Wq (128, 512)
C_WK = 1024     # Wk
C_WV = 1536     # Wv
C_WO = 2048     # Wo chunks (128, 4*128)
C_WE = 2560     # We (rows 0:64, 512)
C_WET = 3072    # WeT (rows 0:64, (h, de) 512)
C_NMY = 3584    # nodes_myT (128, 64)
WPCOLS = 3648

_CACHE = {}


def _install_ntff_hook():
    """antenv.axon_hooks is absent in this image; synthesize it so
    run_bass_kernel_spmd(trace=True) can profile via libaxon."""
    if "antenv.axon_hooks" in sys.modules:
        return
    try:
        from trn_agent_boot.trn_boot import _ntff_profile_via_ctypes
        hook = _ntff_profile_via_ctypes('/opt/axon/libaxon_pjrt.so')
    except Exception:
        hook = None
    mod = types.ModuleType("antenv.axon_hooks")
    mod.get_axon_ntff_profile_hook = lambda: hook
    mod.set_axon_ntff_profile_hook = lambda h: None
    sys.modules["antenv.axon_hooks"] = mod


def _build():
    import concourse.mybir as mybir
    from concourse import bacc
    from concourse.tile import TileContext

    f32 = mybir.dt.float32
    bf = mybir.dt.bfloat16
    nc = bacc.Bacc(None, target_bir_lowering=False)

    wp_d = nc.declare_dram_parameter("wpack", [128, WPCOLS], bf, isOutput=False)
    bq_d = nc.declare_dram_parameter("bq2", [DH, H], f32, isOutput=False)
    enat_d = nc.declare_dram_parameter("edges_nat", [N, NI, DE], bf, isOutput=False)
    etr_d = nc.declare_dram_parameter("edges_T", [128, NP, N], bf, isOutput=False)
    out_d = nc.declare_dram_parameter("out_my", [DN, NI], f32, isOutput=True)

    with TileContext(nc) as tc:
        with contextlib.ExitStack() as ctx:
            const = ctx.enter_context(tc.tile_pool(name="const", bufs=1))
            en_pool = ctx.enter_context(tc.tile_pool(name="edges", bufs=1))
            # PSUM budget is 8 banks; every tile below pads to 1 bank.
            ps_simT = ctx.enter_context(tc.tile_pool(name="ps_simT", bufs=2, space="PSUM"))
            ps_misc = ctx.enter_context(tc.tile_pool(name="ps_misc", bufs=3, space="PSUM"))
            ps_acc = ctx.enter_context(tc.tile_pool(name="ps_acc", bufs=1, space="PSUM"))
            ps_w = ctx.enter_context(tc.tile_pool(name="ps_w", bufs=2, space="PSUM"))

            # ---- one packed weight DMA + bq, then the edge streams ----
            wp = const.tile([128, WPCOLS], bf)
            nc.sync.dma_start(out=wp, in_=wp_d[:, :])
            bq_sb = const.tile([DH, H], f32)
            nc.gpsimd.dma_start(out=bq_sb, in_=bq_d[:, :])

            eT_big = en_pool.tile([128, NP, N], bf, tag="eTb", name="eT_big")
            for g in range(4):
                eng = nc.scalar if g % 2 == 0 else nc.sync
                eng.dma_start(out=eT_big[:, g * 8:(g + 1) * 8, :],
                              in_=etr_d[:, g * 8:(g + 1) * 8, :])
            en_t = [en_pool.tile([128, NI, DE], bf, tag=f"en{jt}", name=f"en{jt}")
                    for jt in range(NJT)]
            for jt in range(NJT):
                eng = nc.sync if jt % 2 == 0 else nc.scalar
                eng.dma_start(out=en_t[jt],
                              in_=enat_d[jt * 128:(jt + 1) * 128, :, :])

            nodesT = wp[:, C_NT:C_NT + N]
            nmyT = wp[:, C_NMY:C_NMY + NI]
            ones = const.tile([128, 128], bf)
            nc.vector.memset(ones, 1.0)

            # ---- projections (qT/u2 chain first: it gates the sim phase) ----
            qT = const.tile([DH, H, NI], bf)          # (dh, h, i) with bq added
            for h in range(H):
                pq = ps_misc.tile([DH, NI], f32, tag="m", name="pq")
                nc.tensor.matmul(out=pq, lhsT=wp[:, C_WQ + h * DH:C_WQ + (h + 1) * DH],
                                 rhs=nmyT, start=True, stop=True)
                nc.vector.tensor_scalar_add(out=qT[:, h, :], in0=pq, scalar1=bq_sb[:, h:h + 1])
            # u2: paired block-diag u (128 = 2x64 de, 32 pairs, 16 = 2x8 h)
            u2 = const.tile([128, NP, 16], bf)
            nc.gpsimd.memset(u2, 0.0)
            for h in range(H):
                pu = ps_misc.tile([DE, NI], f32, tag="m", name="pu")
                nc.tensor.matmul(out=pu, lhsT=wp[0:DE, C_WET + h * DH:C_WET + (h + 1) * DH],
                                 rhs=qT[:, h, :], start=True, stop=True)
                puv = pu.rearrange("d (p two) -> d p two", two=2)
                nc.vector.tensor_copy(out=u2[0:DE, :, h], in_=puv[:, :, 0])
                nc.vector.tensor_copy(out=u2[DE:128, :, 8 + h], in_=puv[:, :, 1])
            kT = const.tile([DH, H, N], bf)           # (dh, h, j)
            for h in range(H):
                pk = ps_misc.tile([DH, N], f32, tag="m", name="pk")
                nc.tensor.matmul(out=pk, lhsT=wp[:, C_WK + h * DH:C_WK + (h + 1) * DH],
                                 rhs=nodesT, start=True, stop=True)
                nc.scalar.copy(out=kT[:, h, :], in_=pk)
            v4 = const.tile([128, NJT, INNER], bf)    # (j, jt, (h dh))
            for jt in range(NJT):
                pv = ps_misc.tile([128, INNER], f32, tag="m", name="pv")
                nc.tensor.matmul(out=pv, lhsT=nodesT[:, jt * 128:(jt + 1) * 128],
                                 rhs=wp[:, C_WV:C_WV + INNER], start=True, stop=True)
                nc.scalar.copy(out=v4[:, jt, :], in_=pv)

            # ---- logits simT (j, i, h) per j-tile; exp in column chunks ----
            expT = const.tile([128, NJT, NI * H], bf)
            for jt in range(NJT):
                simT = ps_simT.tile([128, NI, H], f32, tag="simT", name="simT")
                for h in range(H):
                    nc.tensor.matmul(out=simT[:, :, h],
                                     lhsT=kT[:, h, jt * 128:(jt + 1) * 128],
                                     rhs=qT[:, h, :],
                                     start=(h == 0), stop=False, skip_group_check=True)
                for ip in range(NP):
                    nc.tensor.matmul(out=simT[:, 2 * ip:2 * ip + 2, :],
                                     lhsT=eT_big[:, ip, jt * 128:(jt + 1) * 128],
                                     rhs=u2[:, ip, :],
                                     start=False, stop=(ip == NP - 1), skip_group_check=True)
                for c in range(4):
                    nc.scalar.activation(out=expT[:, jt, c * 128:(c + 1) * 128],
                                         in_=simT[:, 16 * c:16 * (c + 1), :],
                                         func=mybir.ActivationFunctionType.Exp, scale=SCALE)

            # ---- chunked softmax denominators + attnT ----
            recip = const.tile([128, NI * H], f32)
            recip_bf = const.tile([128, NI * H], bf)
            attnT = const.tile([128, NJT, NI * H], bf)
            for c in range(4):
                cs = slice(c * 128, (c + 1) * 128)
                den = ps_misc.tile([128, 128], f32, tag="m", name=f"den{c}")
                for jt in range(NJT):
                    nc.tensor.matmul(out=den, lhsT=ones, rhs=expT[:, jt, cs],
                                     start=(jt == 0), stop=(jt == NJT - 1),
                                     skip_group_check=True)
                nc.vector.reciprocal(out=recip[:, cs], in_=den)
                nc.gpsimd.tensor_copy(out=recip_bf[:, cs], in_=recip[:, cs])
                for jt in range(NJT):
                    (nc.vector if jt % 2 == 0 else nc.gpsimd).tensor_mul(
                        out=attnT[:, jt, cs], in0=expT[:, jt, cs], in1=recip_bf[:, cs])

            attv = attnT.rearrange("p t (i h) -> p t i h", h=H)

            # ---- w[i] = attn[i] @ edges[i] (pair-batched; off-diag garbage) ----
            # chunk order so w starts as soon as its attnT columns are ready
            w_sb = const.tile([DE, NI, H], bf)
            wv2 = w_sb.rearrange("d (i2 two) h -> d i2 two h", two=2)
            for g in range(8):
                pw = ps_w.tile([128, 4, 16], f32, tag="pw", name="pw")
                for pi in range(4):
                    ip = g * 4 + pi
                    for jt in range(NJT):
                        nc.tensor.matmul(out=pw[:, pi, :],
                                         lhsT=en_t[jt][:, 2 * ip:2 * ip + 2, :],
                                         rhs=attv[:, jt, 2 * ip:2 * ip + 2, :],
                                         start=(pi == 0 and jt == 0),
                                         stop=(pi == 3 and jt == NJT - 1),
                                         skip_group_check=True)
                nc.vector.tensor_copy(out=wv2[:, 4 * g:4 * g + 4, 0, :],
                                      in_=pw[0:DE, :, 0:8])
                nc.vector.tensor_copy(out=wv2[:, 4 * g:4 * g + 4, 1, :],
                                      in_=pw[DE:128, :, 8:16])

            # ---- attn @ v and the w @ We_h term, one output psum bank ----
            pout = ps_acc.tile([DH, H, NI], f32)
            for h in range(H):
                for jt in range(NJT):
                    nc.tensor.matmul(out=pout[:, h, :],
                                     lhsT=v4[:, jt, h * DH:(h + 1) * DH],
                                     rhs=attv[:, jt, :, h],
                                     start=(h == 0 and jt == 0), stop=False,
                                     skip_group_check=True)
            for h in range(H):
                nc.tensor.matmul(out=pout[:, h, :],
                                 lhsT=wp[0:DE, C_WE + h * DH:C_WE + (h + 1) * DH],
                                 rhs=w_sb[:, :, h],
                                 start=False, stop=(h == H - 1),
                                 skip_group_check=True)

            # ---- final projection; output stays transposed (dn, i) ----
            oiT = const.tile([128, 4, NI], bf)        # ((h dh) chunk, c, i)
            for h in range(H):
                dst = oiT[(h % 2) * DH:(h % 2) * DH + DH, h // 2, :]
                if h % 2 == 0:
                    nc.vector.tensor_copy(out=dst, in_=pout[:, h, :])
                else:
                    nc.scalar.copy(out=dst, in_=pout[:, h, :])
            pfin = ps_misc.tile([DN, NI], f32, tag="m", name="pfin")
            for c in range(4):
                nc.tensor.matmul(out=pfin, lhsT=wp[:, C_WO + c * 128:C_WO + (c + 1) * 128],
                                 rhs=oiT[:, c, :],
                                 start=(c == 0), stop=(c == 3), skip_group_check=True)
            fin_sb = const.tile([DN, NI], f32)
            nc.vector.tensor_copy(out=fin_sb, in_=pfin)
            nc.gpsimd.dma_start(out=out_d[:, :], in_=fin_sb)

    nc.finalize()
    return nc


def kernel(nodes, edges, mask, Wq, bq, Wk, bk, Wv, bv, We, be, Wo, bo):
    from concourse.bass_utils import run_bass_kernel_spmd

    nodes = np.asarray(nodes, np.float32)
    edges = np.asarray(edges, np.float32)
    mask = np.asarray(mask)
    Wq = np.asarray(Wq, np.float32); bq = np.asarray(bq, np.float32)
    Wk = np.asarray(Wk, np.float32)
    Wv = np.asarray(Wv, np.float32); bv = np.asarray(bv, np.float32)
    We = np.asarray(We, np.float32); be = np.asarray(be, np.float32)
    Wo = np.asarray(Wo, np.float32); bo = np.asarray(bo, np.float32)
    assert mask.all(), "kernel assumes an all-true mask (spec fill=ones)"

    if "nc" not in _CACHE:
        _CACHE["nc"] = _build()
    nc = _CACHE["nc"]

    n0 = nodes[0].astype(BF16)
    e_bf = edges[0].astype(BF16)

    wp_base = np.zeros((128, WPCOLS), BF16)
    wp_base[:, C_NT:C_NT + N] = n0.T
    wp_base[:, C_WQ:C_WQ + INNER] = Wq.astype(BF16)
    wp_base[:, C_WK:C_WK + INNER] = Wk.astype(BF16)
    wp_base[:, C_WV:C_WV + INNER] = Wv.astype(BF16)
    wp_base[:, C_WO:C_WO + INNER] = (
        Wo.reshape(4, 128, DN).transpose(1, 0, 2).reshape(128, 4 * DN).astype(BF16))
    wp_base[0:DE, C_WE:C_WE + INNER] = We.astype(BF16)
    wp_base[0:DE, C_WET:C_WET + INNER] = (
        We.reshape(DE, H, DH).transpose(2, 1, 0).reshape(DH, H * DE).astype(BF16))
    bq2 = np.ascontiguousarray(bq.reshape(H, DH).T)

    in_maps = []
    for c in range(NCORES):
        sl = e_bf[c * NI:(c + 1) * NI]
        wp = wp_base.copy()
        wp[:, C_NMY:C_NMY + NI] = n0[c * NI:(c + 1) * NI].T
        in_maps.append({
            "wpack": wp,
            "bq2": bq2,
            "edges_nat": np.ascontiguousarray(sl.transpose(1, 0, 2)),
            "edges_T": np.ascontiguousarray(
                sl.transpose(0, 2, 1).reshape(NP, 2, DE, N)
                .transpose(1, 2, 0, 3).reshape(128, NP, N)),
        })

    trace = bool(os.environ.get("BASS_KERNEL_TRACE"))
    kw = {}
    if trace:
        _install_ntff_hook()
        import concourse.bass_utils as bu
        bu.upload_artifacts = lambda tmpdir: "local://skipped"
        kw = dict(trace=True, tmpdir=os.environ.get("BASS_KERNEL_TRACE_DIR") or None)
    res = run_bass_kernel_spmd(nc, in_maps, list(range(NCORES)), **kw)
    _CACHE["last_exec_ns"] = res.exec_time_ns

    out = np.concatenate([res.results[c]["out_my"].T for c in range(NCORES)], axis=0)
    out = out + ((be + bv) @ Wo + bo)[None, :]
    return out.reshape(B, N, DN).astype(np.float32)
